# revision 60
# baseline (speedup 1.0000x reference)
"""Bahdanau attention kernel for Trainium2 (8 NeuronCores).

Reference computation (B=32, S=2048, D=1024):
    x      = concat([broadcast(hidden), encoder_outputs], -1)   # [B,S,2D]
    energy = tanh(x @ W + b)                                    # [B,S,D]
    scores = energy . v                                         # [B,S]
    attn   = softmax(mask(scores, src_len))                     # [B,1,S]

Key transformations:
  * x @ W = encoder_outputs @ W[D:] + (hidden @ W[:D]);  the hidden part is
    a tiny per-batch bias vector computed on the host and folded into the
    tanh's per-partition bias on the device.
  * rows with s >= src_len[b] are fully masked out of the softmax, so they
    are never computed: the host packs only the valid rows (padded to SUB
    per batch), load-balances batches across the 8 cores, and the device
    runs a dense kernel on the packed rows.
  * the big E @ W_e matmul runs in fp8 e4m3 with the DoubleRow perf mode
    (two K-planes per cycle -> 2x the f32r MAC rate).  W_e is pre-scaled
    by 32 on the host so its entries use the e4m3 normal range; the 1/32
    is folded into the tanh activation's input scale.  The quantization
    error lands at ~1.2e-2 on the final softmax (gate: 2e-2).
  * the device computes energy^T tiles [D_out=128, rows] in PSUM
    (W_e stationary, host-pre-transposed E^T streaming), applies
    tanh(+bias) on the scalar engine, and reduces against v with an M=1
    f32r matmul back into PSUM.  Masking + softmax run on the host
    (cheap, O(B*S)) because the packed segment boundaries differ per
    core.
"""

import os
import sys

import numpy as np

for _p in ("/root/.axon_site/_ro/trn_rl_repo", "/opt/trn_rl_repo"):
    if os.path.isdir(_p) and _p not in sys.path:
        sys.path.append(_p)

B, S, D = 32, 2048, 1024
N_CORES = 8
# Per-batch row padding granularity == tanh bias subtile width.  256 wastes
# more rows than 128 (~5% vs ~2.5%) but halves the scalar engine's
# per-instruction overhead count: at 128 the ACT drain rate (~1032ns per
# 128x512 m-chunk) falls behind the PE fill rate (~862ns) and throttles the
# whole pipeline via PSUM-bank recycling.
SUB = 256
RB = 512  # main matmul row tile (PSUM free dim)
KO = D // 128  # K chunks (8)
MO = D // 128  # D_out chunks (8)

_NC_CACHE = {}


def _ensure_trace_support():
    """Make trace=True / BASS_TRACE=1 runs survive on images where
    ``antenv.axon_hooks`` is absent (the boot shim degrades silently but
    ``bass_utils`` imports it unconditionally) and where artifact uploads
    to remote storage are unavailable.  No-ops when everything exists."""
    import types

    try:
        import antenv

        try:
            import antenv.axon_hooks  # noqa: F401
        except ImportError:
            mod = types.ModuleType("antenv.axon_hooks")
            state = {"hook": None}
            mod.set_axon_ntff_profile_hook = lambda h: state.__setitem__("hook", h)
            mod.get_axon_ntff_profile_hook = lambda: state["hook"]
            sys.modules["antenv.axon_hooks"] = mod
            antenv.axon_hooks = mod
            try:
                from trn_agent_boot.trn_boot import _ntff_profile_via_ctypes

                so = "/opt/axon/libaxon_pjrt.so"
                if os.path.exists(so):
                    mod.set_axon_ntff_profile_hook(_ntff_profile_via_ctypes(so))
            except Exception:
                pass
    except Exception:
        pass
    try:
        import concourse.bass_utils as bu

        orig = bu.upload_artifacts
        if not getattr(orig, "_safe_wrapped", False):

            def _safe_upload(tmpdir, _orig=orig):
                try:
                    return _orig(tmpdir)
                except Exception:
                    return f"local://{tmpdir}"

            _safe_upload._safe_wrapped = True
            bu.upload_artifacts = _safe_upload
    except Exception:
        pass


def _row_tiles(R):
    """Row-tile sizes covering R rows: 512-tiles plus a 256 tail.

    Returns (sizes, row_offsets) in processing order."""
    assert R % SUB == 0
    sizes = [RB] * (R // RB)
    offs = [i * RB for i in range(len(sizes))]
    if R % RB:
        sizes.append(R % RB)
        offs.append((R // RB) * RB)
    return sizes, offs


def _build_bass(R):
    """Build the per-core SPMD program for R packed rows (R % 256 == 0)."""
    import concourse.bass as bass  # noqa: F401
    import concourse.tile as tile
    from concourse import bacc, mybir

    f32 = mybir.dt.float32
    f32r = mybir.dt.float32r
    f8 = mybir.dt.float8e4
    DR = mybir.MatmulPerfMode.DoubleRow
    n_sub = R // SUB
    tiles, row_offs = _row_tiles(R)

    nc = bacc.Bacc()
    et_d = nc.dram_tensor("et", [D, R], f8, kind="ExternalInput")
    w_d = nc.dram_tensor("wt", [D, D], f8, kind="ExternalInput")
    v_d = nc.dram_tensor("vt", [D], f32, kind="ExternalInput")
    ones_d = nc.dram_tensor("ot", [128, 1], f32r, kind="ExternalInput")
    b_d = nc.dram_tensor("bt", [D, n_sub], f32, kind="ExternalInput")
    out_d = nc.dram_tensor("scores", [1, R], f32, kind="ExternalOutput")

    et_ap = et_d[:, :].rearrange("(ko p) r -> p ko r", p=128)
    w_ap = w_d[:, :].rearrange("(ko p) j -> p ko j", p=128)
    v_ap = v_d[:].rearrange("(mo p) -> p mo", p=128)
    b_ap = b_d[:, :].rearrange("(mo p) s -> p mo s", p=128)

    with tile.TileContext(nc) as tc:
        with (
            tc.tile_pool(name="singles", bufs=1) as singles,
            tc.tile_pool(name="warm", bufs=1) as warm,
            tc.tile_pool(name="et0", bufs=1) as et0_pool,
            tc.tile_pool(name="et", bufs=6) as et_pool,
            tc.tile_pool(name="tanh", bufs=8) as tanh_pool,
            tc.tile_pool(name="acc", bufs=3) as acc_pool,
            tc.tile_pool(name="sc", bufs=2) as sc_pool,
            tc.tile_pool(name="psum_e", bufs=7, space="PSUM") as psum_e,
            tc.tile_pool(name="psum_s", bufs=1, space="PSUM") as psum_s,
        ):
            # --- warmup: keep PE busy + load the ACT tanh table while the
            # first real DMAs are in flight (HAM un-throttles after ~3.4us
            # of PE activity; the ACT table load costs ~2.7us once).  The
            # dummy matmul chain ramps the PE p-state AND triggers the HAM
            # DMA un-throttle while the first W/E tiles stream in.
            # ultra-early PE activity: chain tiny matmuls on the const APs
            # that the runtime preamble loads anyway (~0.5us in), so the HAM
            # DMA un-throttle clock starts ~3us before the memset-fed warmup
            # below can.
            c1 = nc.const_aps.tensor(1.0, (128, 1), mybir.dt.float32)
            wps0 = psum_e.tile([128, RB], f32, tag="ps")
            for _ in range(40):
                nc.tensor.matmul(
                    wps0[0:1, 0:1], c1, c1, start=True, stop=True
                )
            wact = warm.tile([128, 2], f32)
            nc.vector.memset(wact[:], 0.0)
            nc.scalar.activation(
                out=wact[:, 1:2],
                in_=wact[:, 0:1],
                func=mybir.ActivationFunctionType.Tanh,
                bias=0.0,
                scale=1.0,
            )
            wdum = warm.tile([128, RB], f32)
            nc.vector.memset(wdum[:], 0.0)
            wps = psum_e.tile([128, RB], f32, tag="ps")
            for _ in range(5):
                nc.tensor.matmul(
                    wps[:, :],
                    wdum[:, 0:128].bitcast(f32r),
                    wdum[:, :].bitcast(f32r),
                    start=True,
                    stop=True,
                )

            # --- E^T row-block loads: two half-K DMAs per block so
            # dependencies unblock earlier.  Issued with a 2-block
            # prefetch depth; the first block is issued BEFORE the W
            # chunks so the pipeline can start as early as possible.

            et_tiles = {}

            def issue_et(rb):
                NT, r0 = tiles[rb], row_offs[rb]
                et_lo = et_pool.tile([128, KO // 2, RB], f8, tag="etl")
                et_hi = et_pool.tile([128, KO // 2, RB], f8, tag="eth")
                nc.sync.dma_start(
                    out=et_lo[:, :, :NT], in_=et_ap[:, : KO // 2, r0 : r0 + NT]
                )
                nc.sync.dma_start(
                    out=et_hi[:, :, :NT], in_=et_ap[:, KO // 2 :, r0 : r0 + NT]
                )
                et_tiles[rb] = (et_lo, et_hi)

            # --- rb 0 inputs: per-K-chunk E^T tiles interleaved with the W
            # chunk loads, so the first row block can compute k-progressively
            # while the 4MB of W is still arriving from HBM.
            NT0 = tiles[0]
            et0_p = []
            w_p = []
            for kp in range(KO // 2):
                t = et0_pool.tile([128, 2, RB], f8, tag=f"et0_{kp}")
                wk = singles.tile([128, 2, D], f8, tag=f"w{kp}")
                if kp == 0:
                    # first chunk split into singles: the very first matmul
                    # gates on less data
                    for kk in range(2):
                        nc.sync.dma_start(
                            out=t[:, kk, :NT0], in_=et_ap[:, kk, 0:NT0]
                        )
                        nc.sync.dma_start(
                            out=wk[:, kk, :], in_=w_ap[:, kk, :]
                        )
                else:
                    nc.sync.dma_start(
                        out=t[:, :, :NT0],
                        in_=et_ap[:, 2 * kp : 2 * kp + 2, 0:NT0],
                    )
                    nc.sync.dma_start(
                        out=wk[:], in_=w_ap[:, 2 * kp : 2 * kp + 2, :]
                    )
                et0_p.append(t)
                w_p.append(wk)
            v_sb = singles.tile([128, MO], f32)
            nc.sync.dma_start(out=v_sb[:], in_=v_ap)
            bias_sb = singles.tile([128, MO, n_sub], f32)
            nc.sync.dma_start(out=bias_sb[:], in_=b_ap)
            ones_sb = singles.tile([128, 1], f32r)
            nc.sync.dma_start(out=ones_sb[:], in_=ones_d[:, :])

            for _rb in (1, 2, 3, 4, 5):
                if len(tiles) > _rb:
                    issue_et(_rb)

            def act_and_vdot(ps, accs, m, NT, row0):
                # tanh on the scalar engine (per-SUB bias), then fold v in:
                #   acc += tanh(...) * v[m-chunk].
                # The serial acc chain paces the pipeline (~900ns/FMA), so it
                # is split into TWO independent even/odd chains (gpsimd cannot
                # run TensorScalarPtr on core v3, so both stay on DVE).  The PE sums the two acc tiles in the per-tile
                # ones-matmul epilogue.
                # f32r tanh output: the ACT engine writes bf16 ~60% slower,
                # and the ones-matmul needs a single-pass dtype anyway.
                th = tanh_pool.tile([128, RB], f32r)
                for h in range(NT // SUB):
                    s_idx = row0 // SUB + h
                    nc.scalar.activation(
                        out=th[:, h * SUB : (h + 1) * SUB],
                        in_=ps[:, h * SUB : (h + 1) * SUB],
                        func=mybir.ActivationFunctionType.Tanh,
                        bias=bias_sb[:, m, s_idx : s_idx + 1],
                        scale=1.0 / 32.0,
                    )
                eng = nc.vector
                acc = accs[m % 2]
                if m < 2:
                    eng.tensor_scalar_mul(acc[:, :NT], th[:, :NT], v_sb[:, m : m + 1])
                else:
                    eng.scalar_tensor_tensor(
                        out=acc[:, :NT],
                        in0=th[:, :NT],
                        scalar=v_sb[:, m : m + 1],
                        in1=acc[:, :NT],
                        op0=mybir.AluOpType.mult,
                        op1=mybir.AluOpType.add,
                    )

            # Row-tile epilogues (ones-matmul -> copy -> DMA) are deferred by
            # one tile: the ones-matmul depends on the tile's full ACT+DVE
            # chain, which completes while the NEXT tile's E-matmuls run.
            # Emitting it inside the next tile's matmul stream keeps the
            # in-order PE queue from stalling on it.
            pending = []

            def flush_pending():
                while pending:
                    accs_p, NT_p, row0_p = pending.pop(0)
                    sc_ps = psum_s.tile([1, RB], f32)
                    for j in range(2):
                        nc.tensor.matmul(
                            sc_ps[:, :NT_p],
                            ones_sb[:, 0:1],
                            accs_p[j][:, :NT_p],
                            start=(j == 0),
                            stop=(j == 1),
                        )
                    sc_sb = sc_pool.tile([1, RB], f32)
                    nc.vector.tensor_copy(sc_sb[:, :NT_p], sc_ps[:, :NT_p])
                    nc.sync.dma_start(
                        out=out_d[0:1, row0_p : row0_p + NT_p],
                        in_=sc_sb[:, :NT_p],
                    )

            for rb, NT in enumerate(tiles):
                row0 = row_offs[rb]
                if rb + 6 < len(tiles):
                    issue_et(rb + 6)
                accs = (
                    acc_pool.tile([128, RB], f32r, name="acc_e", tag="acc_e"),
                    acc_pool.tile([128, RB], f32r, name="acc_o", tag="acc_o"),
                )
                if rb == 0:
                    # k-pair-outer halves: 4 open PSUM banks accumulate while
                    # the (w_kp, et0_kp) chunk pairs stream in.
                    for half in range(2):
                        ps_l = [
                            psum_e.tile(
                                [128, RB], f32, name=f"ps0_{half}_{mi}", tag="ps"
                            )
                            for mi in range(4)
                        ]
                        for kp in range(KO // 2):
                            for mi in range(4):
                                m = half * 4 + mi
                                nc.tensor.matmul(
                                    ps_l[mi][:, :NT],
                                    w_p[kp][:, :, m * 128 : (m + 1) * 128],
                                    et0_p[kp][:, :, :NT],
                                    start=(kp == 0),
                                    stop=(kp == KO // 2 - 1),
                                    perf_mode=DR,
                                )
                        for mi in range(4):
                            act_and_vdot(ps_l[mi], accs, half * 4 + mi, NT, row0)
                else:
                    et_half = et_tiles.pop(rb)
                    for m in range(MO):
                        ps = psum_e.tile([128, RB], f32, tag="ps")
                        for kp in range(KO // 2):
                            src = et_half[kp // 2]
                            ko2 = (kp % 2) * 2
                            nc.tensor.matmul(
                                ps[:, :NT],
                                w_p[kp][:, :, m * 128 : (m + 1) * 128],
                                src[:, ko2 : ko2 + 2, :NT],
                                start=(kp == 0),
                                stop=(kp == KO // 2 - 1),
                                perf_mode=DR,
                            )
                        if m == 1:
                            flush_pending()
                        act_and_vdot(ps, accs, m, NT, row0)
                pending.append((accs, NT, row0))
            flush_pending()
    nc.compile()
    return nc


def _plan(src_len):
    """Pack valid rows (padded to SUB per batch) and balance across cores.

    LPT greedy followed by a move/swap local search: every core executes the
    same SPMD program over R = max(load) rows, so shaving the max load
    directly shaves kernel time."""
    lens = np.clip(np.asarray(src_len).astype(np.int64), 1, S)
    pads = ((lens + SUB - 1) // SUB) * SUB
    order = np.argsort(-pads, kind="stable")
    loads = [0] * N_CORES
    core_batches = [[] for _ in range(N_CORES)]
    for b in order:
        c = min(range(N_CORES), key=lambda k: loads[k])
        loads[c] += int(pads[b])
        core_batches[c].append(int(b))
    for _ in range(64):
        hi = max(range(N_CORES), key=lambda k: loads[k])
        best = None  # (new_max_pair, kind, payload)
        cur = (loads[hi], -min(loads))
        for lo in range(N_CORES):
            if lo == hi:
                continue
            for b in core_batches[hi]:
                nh, nl = loads[hi] - pads[b], loads[lo] + pads[b]
                cand = (max(nh, nl), -min(nh, nl))
                if cand < cur:
                    cur, best = cand, ("move", b, lo)
            for b in core_batches[hi]:
                for b2 in core_batches[lo]:
                    if pads[b] <= pads[b2]:
                        continue
                    nh = loads[hi] - pads[b] + pads[b2]
                    nl = loads[lo] + pads[b] - pads[b2]
                    cand = (max(nh, nl), -min(nh, nl))
                    if cand < cur:
                        cur, best = cand, ("swap", b, b2, lo)
        if best is None:
            break
        if best[0] == "move":
            _, b, lo = best
            core_batches[hi].remove(b)
            core_batches[lo].append(b)
        else:
            _, b, b2, lo = best
            core_batches[hi].remove(b)
            core_batches[lo].remove(b2)
            core_batches[hi].append(b2)
            core_batches[lo].append(b)
        loads = [int(sum(pads[b] for b in cb)) for cb in core_batches]
    r_max = max(loads)
    R = ((r_max + SUB - 1) // SUB) * SUB
    # layout: per core, list of (batch, row_offset, valid_len, padded_len)
    layout = []
    for c in range(N_CORES):
        cur = 0
        segs = []
        for b in core_batches[c]:
            segs.append((b, cur, int(lens[b]), int(pads[b])))
            cur += int(pads[b])
        layout.append(segs)
    return R, layout


def _run(inputs, trace=False):
    if trace or os.environ.get("BASS_TRACE"):
        _ensure_trace_support()
    from concourse.bass_utils import run_bass_kernel_spmd

    hidden = np.ascontiguousarray(np.asarray(inputs["hidden"]), dtype=np.float32)
    enc = np.asarray(inputs["encoder_outputs"])
    W = np.ascontiguousarray(np.asarray(inputs["W"]), dtype=np.float32)
    bvec = np.ascontiguousarray(np.asarray(inputs["b"]), dtype=np.float32)
    v = np.ascontiguousarray(np.asarray(inputs["v"]), dtype=np.float32)
    src_len = np.asarray(inputs["src_len"])

    import ml_dtypes

    f8 = ml_dtypes.float8_e4m3

    # host-side: per-batch bias = hidden @ W[:D] + b   (0.4% of the FLOPs)
    bias_all = ((hidden @ W[:D]) + bvec[None, :]).astype(np.float32)  # [B, D]
    # W_e pre-scaled by 32 so its N(0, 1/32) entries use the e4m3 normal
    # range; the tanh activation applies 1/32 to the PSUM result.
    w_e8 = np.ascontiguousarray(W[D:] * np.float32(32.0)).astype(f8)  # [D, D]

    R, layout = _plan(src_len)
    n_sub = R // SUB

    in_maps = []
    for c in range(N_CORES):
        et = np.zeros((D, R), dtype=np.float32)
        bt = np.zeros((D, n_sub), dtype=np.float32)
        for b, off, ln, pad in layout[c]:
            et[:, off : off + ln] = np.asarray(enc[b, :ln, :], dtype=np.float32).T
            bt[:, off // SUB : (off + pad) // SUB] = bias_all[b][:, None]
        in_maps.append(
            {
                "et": et.astype(f8),
                "wt": w_e8,
                "vt": v,
                "bt": bt,
                "ot": np.ones((128, 1), dtype=np.float32),
            }
        )

    if R not in _NC_CACHE:
        _NC_CACHE[R] = _build_bass(R)
    nc = _NC_CACHE[R]

    res = run_bass_kernel_spmd(nc, in_maps, list(range(N_CORES)), trace=trace)

    attn = np.zeros((B, 1, S), dtype=np.float32)
    for c in range(N_CORES):
        sc = res.results[c]["scores"][0]
        for b, off, ln, _pad in layout[c]:
            srow = sc[off : off + ln].astype(np.float32)
            m = srow.max()
            e = np.exp(srow - m, dtype=np.float32)
            attn[b, 0, :ln] = e / e.sum(dtype=np.float32)
    return attn, res


def kernel(**inputs):
    attn, _ = _run(inputs, trace=False)
    return attn



# revision 61
# speedup vs baseline: 1.0070x; 1.0070x over previous
"""Bahdanau attention kernel for Trainium2 (8 NeuronCores).

Reference computation (B=32, S=2048, D=1024):
    x      = concat([broadcast(hidden), encoder_outputs], -1)   # [B,S,2D]
    energy = tanh(x @ W + b)                                    # [B,S,D]
    scores = energy . v                                         # [B,S]
    attn   = softmax(mask(scores, src_len))                     # [B,1,S]

Key transformations:
  * x @ W = encoder_outputs @ W[D:] + (hidden @ W[:D]);  the hidden part is
    a tiny per-batch bias vector computed on the host and folded into the
    tanh's per-partition bias on the device.
  * rows with s >= src_len[b] are fully masked out of the softmax, so they
    are never computed: the host packs only the valid rows (padded to SUB
    per batch), load-balances batches across the 8 cores (LPT + swap local
    search -- every core executes the same SPMD program over R = max core
    load rows), and the device runs a dense kernel on the packed rows.
  * the big E @ W_e matmul runs in fp8 e4m3 with the DoubleRow perf mode
    (two K-planes per cycle -> 2x the f32r MAC rate).  W_e is pre-scaled
    by 32 on the host so its entries use the e4m3 normal range; the 1/32
    is folded into the tanh activation's input scale.  The quantization
    error lands at ~1.2e-2 on the final softmax (gate: 2e-2).
  * the device computes energy^T tiles [D_out=128, rows] in PSUM
    (W_e stationary, host-pre-transposed E^T streaming), applies
    tanh(+bias) on the scalar engine (SUB=256-wide instructions -- at 128
    the ACT engine's per-instruction overhead throttles the pipeline), and
    folds v in on the vector engine as two independent even/odd fused
    multiply-add chains (one serial chain paces the whole pipeline).  A
    deferred per-tile ones-matmul (emitted inside the NEXT tile's matmul
    stream so the in-order PE queue never stalls on it) reduces the two
    acc tiles to the row scores.  Masking + softmax run on the host
    (cheap, O(B*S)) because the packed segment boundaries differ per core.
  * startup: a chain of tiny matmuls on the runtime's own const APs plus a
    memset-fed dummy chain puts the PE to work ~0.5us in, which starts the
    HAM DMA un-throttle clock while the first W/E^T tiles stream in.
"""

import os
import sys

import numpy as np

for _p in ("/root/.axon_site/_ro/trn_rl_repo", "/opt/trn_rl_repo"):
    if os.path.isdir(_p) and _p not in sys.path:
        sys.path.append(_p)

B, S, D = 32, 2048, 1024
N_CORES = 8
# Per-batch row padding granularity == tanh bias subtile width.  256 wastes
# more rows than 128 (~5% vs ~2.5%) but halves the scalar engine's
# per-instruction overhead count: at 128 the ACT drain rate (~1032ns per
# 128x512 m-chunk) falls behind the PE fill rate (~862ns) and throttles the
# whole pipeline via PSUM-bank recycling.
SUB = 256
RB = 512  # main matmul row tile (PSUM free dim)
KO = D // 128  # K chunks (8)
MO = D // 128  # D_out chunks (8)

_NC_CACHE = {}


def _ensure_trace_support():
    """Make trace=True / BASS_TRACE=1 runs survive on images where
    ``antenv.axon_hooks`` is absent (the boot shim degrades silently but
    ``bass_utils`` imports it unconditionally) and where artifact uploads
    to remote storage are unavailable.  No-ops when everything exists."""
    import types

    try:
        import antenv

        try:
            import antenv.axon_hooks  # noqa: F401
        except ImportError:
            mod = types.ModuleType("antenv.axon_hooks")
            state = {"hook": None}
            mod.set_axon_ntff_profile_hook = lambda h: state.__setitem__("hook", h)
            mod.get_axon_ntff_profile_hook = lambda: state["hook"]
            sys.modules["antenv.axon_hooks"] = mod
            antenv.axon_hooks = mod
            try:
                from trn_agent_boot.trn_boot import _ntff_profile_via_ctypes

                so = "/opt/axon/libaxon_pjrt.so"
                if os.path.exists(so):
                    mod.set_axon_ntff_profile_hook(_ntff_profile_via_ctypes(so))
            except Exception:
                pass
    except Exception:
        pass
    try:
        import concourse.bass_utils as bu

        orig = bu.upload_artifacts
        if not getattr(orig, "_safe_wrapped", False):

            def _safe_upload(tmpdir, _orig=orig):
                try:
                    return _orig(tmpdir)
                except Exception:
                    return f"local://{tmpdir}"

            _safe_upload._safe_wrapped = True
            bu.upload_artifacts = _safe_upload
    except Exception:
        pass


def _row_tiles(R):
    """Row-tile sizes covering R rows: 512-tiles plus a 256 tail.

    Returns (sizes, row_offsets) in processing order."""
    assert R % SUB == 0
    sizes = [RB] * (R // RB)
    offs = [i * RB for i in range(len(sizes))]
    if R % RB:
        sizes.append(R % RB)
        offs.append((R // RB) * RB)
    return sizes, offs


def _build_bass(R):
    """Build the per-core SPMD program for R packed rows (R % 256 == 0)."""
    import concourse.bass as bass  # noqa: F401
    import concourse.tile as tile
    from concourse import bacc, mybir

    f32 = mybir.dt.float32
    f32r = mybir.dt.float32r
    f8 = mybir.dt.float8e4
    DR = mybir.MatmulPerfMode.DoubleRow
    n_sub = R // SUB
    tiles, row_offs = _row_tiles(R)

    nc = bacc.Bacc()
    et_d = nc.dram_tensor("et", [D, R], f8, kind="ExternalInput")
    w_d = nc.dram_tensor("wt", [D, D], f8, kind="ExternalInput")
    v_d = nc.dram_tensor("vt", [D], f32, kind="ExternalInput")
    ones_d = nc.dram_tensor("ot", [128, 1], f32r, kind="ExternalInput")
    b_d = nc.dram_tensor("bt", [D, n_sub], f32, kind="ExternalInput")
    out_d = nc.dram_tensor("scores", [1, R], f32, kind="ExternalOutput")

    et_ap = et_d[:, :].rearrange("(ko p) r -> p ko r", p=128)
    w_ap = w_d[:, :].rearrange("(ko p) j -> p ko j", p=128)
    v_ap = v_d[:].rearrange("(mo p) -> p mo", p=128)
    b_ap = b_d[:, :].rearrange("(mo p) s -> p mo s", p=128)

    with tile.TileContext(nc) as tc:
        with (
            tc.tile_pool(name="singles", bufs=1) as singles,
            tc.tile_pool(name="warm", bufs=1) as warm,
            tc.tile_pool(name="et0", bufs=1) as et0_pool,
            tc.tile_pool(name="et", bufs=6) as et_pool,
            tc.tile_pool(name="tanh", bufs=8) as tanh_pool,
            tc.tile_pool(name="acc", bufs=3) as acc_pool,
            tc.tile_pool(name="sc", bufs=2) as sc_pool,
            tc.tile_pool(name="psum_e", bufs=7, space="PSUM") as psum_e,
            tc.tile_pool(name="psum_s", bufs=1, space="PSUM") as psum_s,
        ):
            # --- warmup: keep PE busy + load the ACT tanh table while the
            # first real DMAs are in flight (HAM un-throttles after ~3.4us
            # of PE activity; the ACT table load costs ~2.7us once).  The
            # dummy matmul chain ramps the PE p-state AND triggers the HAM
            # DMA un-throttle while the first W/E tiles stream in.
            # ultra-early PE activity: chain tiny matmuls on the const APs
            # that the runtime preamble loads anyway (~0.5us in), so the HAM
            # DMA un-throttle clock starts ~3us before the memset-fed warmup
            # below can.
            c1 = nc.const_aps.tensor(1.0, (128, 1), mybir.dt.float32)
            wps0 = psum_e.tile([128, RB], f32, tag="ps")
            for _ in range(40):
                nc.tensor.matmul(
                    wps0[0:1, 0:1], c1, c1, start=True, stop=True
                )
            wact = warm.tile([128, 2], f32)
            nc.vector.memset(wact[:], 0.0)
            nc.scalar.activation(
                out=wact[:, 1:2],
                in_=wact[:, 0:1],
                func=mybir.ActivationFunctionType.Tanh,
                bias=0.0,
                scale=1.0,
            )
            wdum = warm.tile([128, RB], f32)
            nc.vector.memset(wdum[:], 0.0)
            wps = psum_e.tile([128, RB], f32, tag="ps")
            for _ in range(5):
                nc.tensor.matmul(
                    wps[:, :],
                    wdum[:, 0:128].bitcast(f32r),
                    wdum[:, :].bitcast(f32r),
                    start=True,
                    stop=True,
                )

            # --- E^T row-block loads: two half-K DMAs per block so
            # dependencies unblock earlier.  Issued with a 2-block
            # prefetch depth; the first block is issued BEFORE the W
            # chunks so the pipeline can start as early as possible.

            et_tiles = {}

            def issue_et(rb):
                NT, r0 = tiles[rb], row_offs[rb]
                et_lo = et_pool.tile([128, KO // 2, RB], f8, tag="etl")
                et_hi = et_pool.tile([128, KO // 2, RB], f8, tag="eth")
                nc.sync.dma_start(
                    out=et_lo[:, :, :NT], in_=et_ap[:, : KO // 2, r0 : r0 + NT]
                )
                nc.sync.dma_start(
                    out=et_hi[:, :, :NT], in_=et_ap[:, KO // 2 :, r0 : r0 + NT]
                )
                et_tiles[rb] = (et_lo, et_hi)

            # --- rb 0 inputs: per-K-chunk E^T tiles interleaved with the W
            # chunk loads, so the first row block can compute k-progressively
            # while the 4MB of W is still arriving from HBM.
            NT0 = tiles[0]
            et0_p = []
            w_p = []
            for kp in range(KO // 2):
                t = et0_pool.tile([128, 2, RB], f8, tag=f"et0_{kp}")
                wk = singles.tile([128, 2, D], f8, tag=f"w{kp}")
                if kp == 0:
                    # first chunk split into singles: the very first matmul
                    # gates on less data
                    for kk in range(2):
                        nc.sync.dma_start(
                            out=t[:, kk, :NT0], in_=et_ap[:, kk, 0:NT0]
                        )
                        nc.sync.dma_start(
                            out=wk[:, kk, :], in_=w_ap[:, kk, :]
                        )
                else:
                    nc.sync.dma_start(
                        out=t[:, :, :NT0],
                        in_=et_ap[:, 2 * kp : 2 * kp + 2, 0:NT0],
                    )
                    nc.sync.dma_start(
                        out=wk[:], in_=w_ap[:, 2 * kp : 2 * kp + 2, :]
                    )
                et0_p.append(t)
                w_p.append(wk)
            v_sb = singles.tile([128, MO], f32)
            nc.sync.dma_start(out=v_sb[:], in_=v_ap)
            bias_sb = singles.tile([128, MO, n_sub], f32)
            nc.sync.dma_start(out=bias_sb[:], in_=b_ap)
            ones_sb = singles.tile([128, 1], f32r)
            nc.sync.dma_start(out=ones_sb[:], in_=ones_d[:, :])

            for _rb in (1, 2, 3, 4, 5):
                if len(tiles) > _rb:
                    issue_et(_rb)

            def act_and_vdot(ps, accs, m, NT, row0):
                # tanh on the scalar engine (per-SUB bias), then fold v in:
                #   acc += tanh(...) * v[m-chunk].
                # The serial acc chain paces the pipeline (~900ns/FMA), so it
                # is split into TWO independent even/odd chains (gpsimd cannot
                # run TensorScalarPtr on core v3, so both stay on DVE).  The PE sums the two acc tiles in the per-tile
                # ones-matmul epilogue.
                # f32r tanh output: the ACT engine writes bf16 ~60% slower,
                # and the ones-matmul needs a single-pass dtype anyway.
                th = tanh_pool.tile([128, RB], f32r)
                for h in range(NT // SUB):
                    s_idx = row0 // SUB + h
                    nc.scalar.activation(
                        out=th[:, h * SUB : (h + 1) * SUB],
                        in_=ps[:, h * SUB : (h + 1) * SUB],
                        func=mybir.ActivationFunctionType.Tanh,
                        bias=bias_sb[:, m, s_idx : s_idx + 1],
                        scale=1.0 / 32.0,
                    )
                eng = nc.vector
                acc = accs[m % 2]
                if m < 2:
                    eng.tensor_scalar_mul(acc[:, :NT], th[:, :NT], v_sb[:, m : m + 1])
                else:
                    eng.scalar_tensor_tensor(
                        out=acc[:, :NT],
                        in0=th[:, :NT],
                        scalar=v_sb[:, m : m + 1],
                        in1=acc[:, :NT],
                        op0=mybir.AluOpType.mult,
                        op1=mybir.AluOpType.add,
                    )

            # Row-tile epilogues (ones-matmul -> copy -> DMA) are deferred by
            # one tile: the ones-matmul depends on the tile's full ACT+DVE
            # chain, which completes while the NEXT tile's E-matmuls run.
            # Emitting it inside the next tile's matmul stream keeps the
            # in-order PE queue from stalling on it.
            pending = []

            def flush_pending():
                while pending:
                    accs_p, NT_p, row0_p = pending.pop(0)
                    sc_ps = psum_s.tile([1, RB], f32)
                    for j in range(2):
                        nc.tensor.matmul(
                            sc_ps[:, :NT_p],
                            ones_sb[:, 0:1],
                            accs_p[j][:, :NT_p],
                            start=(j == 0),
                            stop=(j == 1),
                        )
                    sc_sb = sc_pool.tile([1, RB], f32)
                    nc.vector.tensor_copy(sc_sb[:, :NT_p], sc_ps[:, :NT_p])
                    nc.sync.dma_start(
                        out=out_d[0:1, row0_p : row0_p + NT_p],
                        in_=sc_sb[:, :NT_p],
                    )

            for rb, NT in enumerate(tiles):
                row0 = row_offs[rb]
                if rb + 6 < len(tiles):
                    issue_et(rb + 6)
                accs = (
                    acc_pool.tile([128, RB], f32r, name="acc_e", tag="acc_e"),
                    acc_pool.tile([128, RB], f32r, name="acc_o", tag="acc_o"),
                )
                if rb == 0:
                    # k-pair-outer halves: 4 open PSUM banks accumulate while
                    # the (w_kp, et0_kp) chunk pairs stream in.
                    for half in range(2):
                        ps_l = [
                            psum_e.tile(
                                [128, RB], f32, name=f"ps0_{half}_{mi}", tag="ps"
                            )
                            for mi in range(4)
                        ]
                        for kp in range(KO // 2):
                            for mi in range(4):
                                m = half * 4 + mi
                                nc.tensor.matmul(
                                    ps_l[mi][:, :NT],
                                    w_p[kp][:, :, m * 128 : (m + 1) * 128],
                                    et0_p[kp][:, :, :NT],
                                    start=(kp == 0),
                                    stop=(kp == KO // 2 - 1),
                                    perf_mode=DR,
                                )
                        for mi in range(4):
                            act_and_vdot(ps_l[mi], accs, half * 4 + mi, NT, row0)
                else:
                    et_half = et_tiles.pop(rb)
                    for m in range(MO):
                        ps = psum_e.tile([128, RB], f32, tag="ps")
                        for kp in range(KO // 2):
                            src = et_half[kp // 2]
                            ko2 = (kp % 2) * 2
                            nc.tensor.matmul(
                                ps[:, :NT],
                                w_p[kp][:, :, m * 128 : (m + 1) * 128],
                                src[:, ko2 : ko2 + 2, :NT],
                                start=(kp == 0),
                                stop=(kp == KO // 2 - 1),
                                perf_mode=DR,
                            )
                        if m == 1:
                            flush_pending()
                        act_and_vdot(ps, accs, m, NT, row0)
                pending.append((accs, NT, row0))
            flush_pending()
    nc.compile()
    return nc


def _plan(src_len):
    """Pack valid rows (padded to SUB per batch) and balance across cores.

    LPT greedy followed by a move/swap local search: every core executes the
    same SPMD program over R = max(load) rows, so shaving the max load
    directly shaves kernel time."""
    lens = np.clip(np.asarray(src_len).astype(np.int64), 1, S)
    pads = ((lens + SUB - 1) // SUB) * SUB
    order = np.argsort(-pads, kind="stable")
    loads = [0] * N_CORES
    core_batches = [[] for _ in range(N_CORES)]
    for b in order:
        c = min(range(N_CORES), key=lambda k: loads[k])
        loads[c] += int(pads[b])
        core_batches[c].append(int(b))
    for _ in range(64):
        hi = max(range(N_CORES), key=lambda k: loads[k])
        best = None  # (new_max_pair, kind, payload)
        cur = (loads[hi], -min(loads))
        for lo in range(N_CORES):
            if lo == hi:
                continue
            for b in core_batches[hi]:
                nh, nl = loads[hi] - pads[b], loads[lo] + pads[b]
                cand = (max(nh, nl), -min(nh, nl))
                if cand < cur:
                    cur, best = cand, ("move", b, lo)
            for b in core_batches[hi]:
                for b2 in core_batches[lo]:
                    if pads[b] <= pads[b2]:
                        continue
                    nh = loads[hi] - pads[b] + pads[b2]
                    nl = loads[lo] + pads[b] - pads[b2]
                    cand = (max(nh, nl), -min(nh, nl))
                    if cand < cur:
                        cur, best = cand, ("swap", b, b2, lo)
        if best is None:
            break
        if best[0] == "move":
            _, b, lo = best
            core_batches[hi].remove(b)
            core_batches[lo].append(b)
        else:
            _, b, b2, lo = best
            core_batches[hi].remove(b)
            core_batches[lo].remove(b2)
            core_batches[hi].append(b2)
            core_batches[lo].append(b)
        loads = [int(sum(pads[b] for b in cb)) for cb in core_batches]
    r_max = max(loads)
    R = ((r_max + SUB - 1) // SUB) * SUB
    # layout: per core, list of (batch, row_offset, valid_len, padded_len)
    layout = []
    for c in range(N_CORES):
        cur = 0
        segs = []
        for b in core_batches[c]:
            segs.append((b, cur, int(lens[b]), int(pads[b])))
            cur += int(pads[b])
        layout.append(segs)
    return R, layout


def _run(inputs, trace=False):
    if trace or os.environ.get("BASS_TRACE"):
        _ensure_trace_support()
    from concourse.bass_utils import run_bass_kernel_spmd

    hidden = np.ascontiguousarray(np.asarray(inputs["hidden"]), dtype=np.float32)
    enc = np.asarray(inputs["encoder_outputs"])
    W = np.ascontiguousarray(np.asarray(inputs["W"]), dtype=np.float32)
    bvec = np.ascontiguousarray(np.asarray(inputs["b"]), dtype=np.float32)
    v = np.ascontiguousarray(np.asarray(inputs["v"]), dtype=np.float32)
    src_len = np.asarray(inputs["src_len"])

    import ml_dtypes

    f8 = ml_dtypes.float8_e4m3

    # host-side: per-batch bias = hidden @ W[:D] + b   (0.4% of the FLOPs)
    bias_all = ((hidden @ W[:D]) + bvec[None, :]).astype(np.float32)  # [B, D]
    # W_e pre-scaled by 32 so its N(0, 1/32) entries use the e4m3 normal
    # range; the tanh activation applies 1/32 to the PSUM result.
    w_e8 = np.ascontiguousarray(W[D:] * np.float32(32.0)).astype(f8)  # [D, D]

    R, layout = _plan(src_len)
    n_sub = R // SUB

    in_maps = []
    for c in range(N_CORES):
        et = np.zeros((D, R), dtype=np.float32)
        bt = np.zeros((D, n_sub), dtype=np.float32)
        for b, off, ln, pad in layout[c]:
            et[:, off : off + ln] = np.asarray(enc[b, :ln, :], dtype=np.float32).T
            bt[:, off // SUB : (off + pad) // SUB] = bias_all[b][:, None]
        in_maps.append(
            {
                "et": et.astype(f8),
                "wt": w_e8,
                "vt": v,
                "bt": bt,
                "ot": np.ones((128, 1), dtype=np.float32),
            }
        )

    if R not in _NC_CACHE:
        _NC_CACHE[R] = _build_bass(R)
    nc = _NC_CACHE[R]

    res = run_bass_kernel_spmd(nc, in_maps, list(range(N_CORES)), trace=trace)

    attn = np.zeros((B, 1, S), dtype=np.float32)
    for c in range(N_CORES):
        sc = res.results[c]["scores"][0]
        for b, off, ln, _pad in layout[c]:
            srow = sc[off : off + ln].astype(np.float32)
            m = srow.max()
            e = np.exp(srow - m, dtype=np.float32)
            attn[b, 0, :ln] = e / e.sum(dtype=np.float32)
    return attn, res


def kernel(**inputs):
    attn, _ = _run(inputs, trace=False)
    return attn



# revision 66
# speedup vs baseline: 1.0142x; 1.0072x over previous
"""Bahdanau attention kernel for Trainium2 (8 NeuronCores).

Reference computation (B=32, S=2048, D=1024):
    x      = concat([broadcast(hidden), encoder_outputs], -1)   # [B,S,2D]
    energy = tanh(x @ W + b)                                    # [B,S,D]
    scores = energy . v                                         # [B,S]
    attn   = softmax(mask(scores, src_len))                     # [B,1,S]

Key transformations:
  * x @ W = encoder_outputs @ W[D:] + (hidden @ W[:D]);  the hidden part is
    a tiny per-batch bias vector computed on the host and folded into the
    tanh's per-partition bias on the device.
  * rows with s >= src_len[b] are fully masked out of the softmax, so they
    are never computed: the host packs only the valid rows (padded to SUB
    per batch), load-balances batches across the 8 cores (LPT + swap local
    search -- every core executes the same SPMD program over R = max core
    load rows), and the device runs a dense kernel on the packed rows.
  * the big E @ W_e matmul runs in fp8 e4m3 with the DoubleRow perf mode
    (two K-planes per cycle -> 2x the f32r MAC rate).  W_e is pre-scaled
    by 32 on the host so its entries use the e4m3 normal range; the 1/32
    is folded into the tanh activation's input scale.  The quantization
    error lands at ~1.2e-2 on the final softmax (gate: 2e-2).
  * the device computes energy^T tiles [D_out=128, rows] in PSUM
    (W_e stationary, host-pre-transposed E^T streaming), applies
    tanh(+bias) on the scalar engine (SUB=256-wide instructions -- at 128
    the ACT engine's per-instruction overhead throttles the pipeline), and
    folds v in on the vector engine as two independent even/odd fused
    multiply-add chains (one serial chain paces the whole pipeline).  A
    deferred per-tile ones-matmul (emitted inside the NEXT tile's matmul
    stream so the in-order PE queue never stalls on it) reduces the two
    acc tiles to the row scores.  Masking + softmax run on the host
    (cheap, O(B*S)) because the packed segment boundaries differ per core.
  * startup: a chain of tiny matmuls on the runtime's own const APs plus a
    memset-fed dummy chain puts the PE to work ~0.5us in, which starts the
    HAM DMA un-throttle clock while the first W/E^T tiles stream in.
"""

import os
import sys

import numpy as np

for _p in ("/root/.axon_site/_ro/trn_rl_repo", "/opt/trn_rl_repo"):
    if os.path.isdir(_p) and _p not in sys.path:
        sys.path.append(_p)

B, S, D = 32, 2048, 1024
N_CORES = 8
# Per-batch row padding granularity == tanh bias subtile width.  256 wastes
# more rows than 128 (~5% vs ~2.5%) but halves the scalar engine's
# per-instruction overhead count: at 128 the ACT drain rate (~1032ns per
# 128x512 m-chunk) falls behind the PE fill rate (~862ns) and throttles the
# whole pipeline via PSUM-bank recycling.
SUB = 256
RB = 512  # main matmul row tile (PSUM free dim)
KO = D // 128  # K chunks (8)
MO = D // 128  # D_out chunks (8)

_NC_CACHE = {}


def _ensure_trace_support():
    """Make trace=True / BASS_TRACE=1 runs survive on images where
    ``antenv.axon_hooks`` is absent (the boot shim degrades silently but
    ``bass_utils`` imports it unconditionally) and where artifact uploads
    to remote storage are unavailable.  No-ops when everything exists."""
    import types

    try:
        import antenv

        try:
            import antenv.axon_hooks  # noqa: F401
        except ImportError:
            mod = types.ModuleType("antenv.axon_hooks")
            state = {"hook": None}
            mod.set_axon_ntff_profile_hook = lambda h: state.__setitem__("hook", h)
            mod.get_axon_ntff_profile_hook = lambda: state["hook"]
            sys.modules["antenv.axon_hooks"] = mod
            antenv.axon_hooks = mod
            try:
                from trn_agent_boot.trn_boot import _ntff_profile_via_ctypes

                so = "/opt/axon/libaxon_pjrt.so"
                if os.path.exists(so):
                    mod.set_axon_ntff_profile_hook(_ntff_profile_via_ctypes(so))
            except Exception:
                pass
    except Exception:
        pass
    try:
        import concourse.bass_utils as bu

        orig = bu.upload_artifacts
        if not getattr(orig, "_safe_wrapped", False):

            def _safe_upload(tmpdir, _orig=orig):
                try:
                    return _orig(tmpdir)
                except Exception:
                    return f"local://{tmpdir}"

            _safe_upload._safe_wrapped = True
            bu.upload_artifacts = _safe_upload
    except Exception:
        pass


def _row_tiles(R):
    """Row-tile sizes covering R rows: 512-tiles plus a 128/256/384 tail.

    Returns (sizes, row_offsets) in processing order."""
    assert R % 128 == 0
    sizes = [RB] * (R // RB)
    offs = [i * RB for i in range(len(sizes))]
    if R % RB:
        sizes.append(R % RB)
        offs.append((R // RB) * RB)
    return sizes, offs


def _build_bass(R):
    """Build the per-core SPMD program for R packed rows (R % 256 == 0)."""
    import concourse.bass as bass  # noqa: F401
    import concourse.tile as tile
    from concourse import bacc, mybir

    f32 = mybir.dt.float32
    f32r = mybir.dt.float32r
    f8 = mybir.dt.float8e4
    DR = mybir.MatmulPerfMode.DoubleRow
    n_sub = (R + SUB - 1) // SUB
    tiles, row_offs = _row_tiles(R)

    nc = bacc.Bacc()
    et_d = nc.dram_tensor("et", [D, R], f8, kind="ExternalInput")
    w_d = nc.dram_tensor("wt", [D, D], f8, kind="ExternalInput")
    v_d = nc.dram_tensor("vt", [D], f32, kind="ExternalInput")
    ones_d = nc.dram_tensor("ot", [128, 1], f32r, kind="ExternalInput")
    b_d = nc.dram_tensor("bt", [D, n_sub], f32, kind="ExternalInput")
    out_d = nc.dram_tensor("scores", [1, R], f32, kind="ExternalOutput")

    et_ap = et_d[:, :].rearrange("(ko p) r -> p ko r", p=128)
    w_ap = w_d[:, :].rearrange("(ko p) j -> p ko j", p=128)
    v_ap = v_d[:].rearrange("(mo p) -> p mo", p=128)
    b_ap = b_d[:, :].rearrange("(mo p) s -> p mo s", p=128)

    with tile.TileContext(nc) as tc:
        with (
            tc.tile_pool(name="singles", bufs=1) as singles,
            tc.tile_pool(name="warm", bufs=1) as warm,
            tc.tile_pool(name="et0", bufs=1) as et0_pool,
            tc.tile_pool(name="et", bufs=6) as et_pool,
            tc.tile_pool(name="tanh", bufs=8) as tanh_pool,
            tc.tile_pool(name="acc", bufs=3) as acc_pool,
            tc.tile_pool(name="sc", bufs=2) as sc_pool,
            tc.tile_pool(name="psum_e", bufs=7, space="PSUM") as psum_e,
            tc.tile_pool(name="psum_s", bufs=1, space="PSUM") as psum_s,
        ):
            # --- warmup: keep PE busy + load the ACT tanh table while the
            # first real DMAs are in flight (HAM un-throttles after ~3.4us
            # of PE activity; the ACT table load costs ~2.7us once).  The
            # dummy matmul chain ramps the PE p-state AND triggers the HAM
            # DMA un-throttle while the first W/E tiles stream in.
            # ultra-early PE activity: chain tiny matmuls on the const APs
            # that the runtime preamble loads anyway (~0.5us in), so the HAM
            # DMA un-throttle clock starts ~3us before the memset-fed warmup
            # below can.
            c1 = nc.const_aps.tensor(1.0, (128, 1), mybir.dt.float32)
            wps0 = psum_e.tile([128, RB], f32, tag="ps")
            for _ in range(40):
                nc.tensor.matmul(
                    wps0[0:1, 0:1], c1, c1, start=True, stop=True
                )
            wact = warm.tile([128, 2], f32)
            nc.vector.memset(wact[:], 0.0)
            nc.scalar.activation(
                out=wact[:, 1:2],
                in_=wact[:, 0:1],
                func=mybir.ActivationFunctionType.Tanh,
                bias=0.0,
                scale=1.0,
            )
            wdum = warm.tile([128, RB], f32)
            nc.vector.memset(wdum[:], 0.0)
            wps = psum_e.tile([128, RB], f32, tag="ps")
            for _ in range(5):
                nc.tensor.matmul(
                    wps[:, :],
                    wdum[:, 0:128].bitcast(f32r),
                    wdum[:, :].bitcast(f32r),
                    start=True,
                    stop=True,
                )

            # --- E^T row-block loads: two half-K DMAs per block so
            # dependencies unblock earlier.  Issued with a 2-block
            # prefetch depth; the first block is issued BEFORE the W
            # chunks so the pipeline can start as early as possible.

            et_tiles = {}

            def issue_et(rb):
                NT, r0 = tiles[rb], row_offs[rb]
                et_lo = et_pool.tile([128, KO // 2, RB], f8, tag="etl")
                et_hi = et_pool.tile([128, KO // 2, RB], f8, tag="eth")
                nc.sync.dma_start(
                    out=et_lo[:, :, :NT], in_=et_ap[:, : KO // 2, r0 : r0 + NT]
                )
                nc.sync.dma_start(
                    out=et_hi[:, :, :NT], in_=et_ap[:, KO // 2 :, r0 : r0 + NT]
                )
                et_tiles[rb] = (et_lo, et_hi)

            # --- rb 0 inputs: per-K-chunk E^T tiles interleaved with the W
            # chunk loads, so the first row block can compute k-progressively
            # while the 4MB of W is still arriving from HBM.
            NT0 = tiles[0]
            et0_p = []
            w_p = []
            for kp in range(KO // 2):
                t = et0_pool.tile([128, 2, RB], f8, tag=f"et0_{kp}")
                wk = singles.tile([128, 2, D], f8, tag=f"w{kp}")
                if kp == 0:
                    # first chunk split into singles: the very first matmul
                    # gates on less data
                    for kk in range(2):
                        nc.sync.dma_start(
                            out=t[:, kk, :NT0], in_=et_ap[:, kk, 0:NT0]
                        )
                        nc.sync.dma_start(
                            out=wk[:, kk, :], in_=w_ap[:, kk, :]
                        )
                else:
                    nc.sync.dma_start(
                        out=t[:, :, :NT0],
                        in_=et_ap[:, 2 * kp : 2 * kp + 2, 0:NT0],
                    )
                    nc.sync.dma_start(
                        out=wk[:], in_=w_ap[:, 2 * kp : 2 * kp + 2, :]
                    )
                et0_p.append(t)
                w_p.append(wk)
            v_sb = singles.tile([128, MO], f32)
            nc.sync.dma_start(out=v_sb[:], in_=v_ap)
            bias_sb = singles.tile([128, MO, n_sub], f32)
            nc.sync.dma_start(out=bias_sb[:], in_=b_ap)
            ones_sb = singles.tile([128, 1], f32r)
            nc.sync.dma_start(out=ones_sb[:], in_=ones_d[:, :])

            for _rb in (1, 2, 3, 4, 5):
                if len(tiles) > _rb:
                    issue_et(_rb)

            def act_and_vdot(ps, accs, m, NT, row0):
                # tanh on the scalar engine (per-SUB bias), then fold v in:
                #   acc += tanh(...) * v[m-chunk].
                # The serial acc chain paces the pipeline (~900ns/FMA), so it
                # is split into TWO independent even/odd chains (gpsimd cannot
                # run TensorScalarPtr on core v3, so both stay on DVE).  The PE sums the two acc tiles in the per-tile
                # ones-matmul epilogue.
                # f32r tanh output: the ACT engine writes bf16 ~60% slower,
                # and the ones-matmul needs a single-pass dtype anyway.
                th = tanh_pool.tile([128, RB], f32r)
                pos = 0
                while pos < NT:
                    w = min(SUB, NT - pos)
                    s_idx = (row0 + pos) // SUB
                    nc.scalar.activation(
                        out=th[:, pos : pos + w],
                        in_=ps[:, pos : pos + w],
                        func=mybir.ActivationFunctionType.Tanh,
                        bias=bias_sb[:, m, s_idx : s_idx + 1],
                        scale=1.0 / 32.0,
                    )
                    pos += w
                eng = nc.vector
                acc = accs[m % 2]
                if m < 2:
                    eng.tensor_scalar_mul(acc[:, :NT], th[:, :NT], v_sb[:, m : m + 1])
                else:
                    eng.scalar_tensor_tensor(
                        out=acc[:, :NT],
                        in0=th[:, :NT],
                        scalar=v_sb[:, m : m + 1],
                        in1=acc[:, :NT],
                        op0=mybir.AluOpType.mult,
                        op1=mybir.AluOpType.add,
                    )

            # Row-tile epilogues (ones-matmul -> copy -> DMA) are deferred by
            # one tile: the ones-matmul depends on the tile's full ACT+DVE
            # chain, which completes while the NEXT tile's E-matmuls run.
            # Emitting it inside the next tile's matmul stream keeps the
            # in-order PE queue from stalling on it.
            pending = []

            def flush_pending():
                while pending:
                    accs_p, NT_p, row0_p = pending.pop(0)
                    sc_ps = psum_s.tile([1, RB], f32)
                    for j in range(2):
                        nc.tensor.matmul(
                            sc_ps[:, :NT_p],
                            ones_sb[:, 0:1],
                            accs_p[j][:, :NT_p],
                            start=(j == 0),
                            stop=(j == 1),
                        )
                    sc_sb = sc_pool.tile([1, RB], f32)
                    nc.vector.tensor_copy(sc_sb[:, :NT_p], sc_ps[:, :NT_p])
                    nc.sync.dma_start(
                        out=out_d[0:1, row0_p : row0_p + NT_p],
                        in_=sc_sb[:, :NT_p],
                    )

            for rb, NT in enumerate(tiles):
                row0 = row_offs[rb]
                if rb + 6 < len(tiles):
                    issue_et(rb + 6)
                accs = (
                    acc_pool.tile([128, RB], f32r, name="acc_e", tag="acc_e"),
                    acc_pool.tile([128, RB], f32r, name="acc_o", tag="acc_o"),
                )
                if rb == 0:
                    # k-pair-outer halves: 4 open PSUM banks accumulate while
                    # the (w_kp, et0_kp) chunk pairs stream in.
                    for half in range(2):
                        ps_l = [
                            psum_e.tile(
                                [128, RB], f32, name=f"ps0_{half}_{mi}", tag="ps"
                            )
                            for mi in range(4)
                        ]
                        for kp in range(KO // 2):
                            for mi in range(4):
                                m = half * 4 + mi
                                nc.tensor.matmul(
                                    ps_l[mi][:, :NT],
                                    w_p[kp][:, :, m * 128 : (m + 1) * 128],
                                    et0_p[kp][:, :, :NT],
                                    start=(kp == 0),
                                    stop=(kp == KO // 2 - 1),
                                    perf_mode=DR,
                                )
                        for mi in range(4):
                            act_and_vdot(ps_l[mi], accs, half * 4 + mi, NT, row0)
                else:
                    et_half = et_tiles.pop(rb)
                    for m in range(MO):
                        ps = psum_e.tile([128, RB], f32, tag="ps")
                        for kp in range(KO // 2):
                            src = et_half[kp // 2]
                            ko2 = (kp % 2) * 2
                            nc.tensor.matmul(
                                ps[:, :NT],
                                w_p[kp][:, :, m * 128 : (m + 1) * 128],
                                src[:, ko2 : ko2 + 2, :NT],
                                start=(kp == 0),
                                stop=(kp == KO // 2 - 1),
                                perf_mode=DR,
                            )
                        if m == 1:
                            flush_pending()
                        act_and_vdot(ps, accs, m, NT, row0)
                pending.append((accs, NT, row0))
            flush_pending()
    nc.compile()
    return nc


def _plan(src_len):
    """Pack valid rows (padded to SUB per batch) and balance across cores.

    Every core executes the same SPMD program over R = max effective load,
    so shaving the max directly shaves kernel time.  The LAST batch on each
    core needs no SUB-rounding (nothing follows it), so a core's effective
    load is sum(pads of all but the max-waste batch) + true length of that
    batch, rounded to the 128-row tile granularity.  LPT greedy on padded
    sizes, then a move/swap local search on the effective objective."""
    lens = np.clip(np.asarray(src_len).astype(np.int64), 1, S)
    pads = ((lens + SUB - 1) // SUB) * SUB
    waste = pads - lens

    def eff(cb):
        if not cb:
            return 0
        load = int(sum(pads[b] for b in cb))
        w = max(int(waste[b]) for b in cb)
        return ((load - w + 127) // 128) * 128

    def score(cbs):
        effs = sorted((eff(cb) for cb in cbs), reverse=True)
        return tuple(effs)

    order = np.argsort(-pads, kind="stable")
    loads = [0] * N_CORES
    core_batches = [[] for _ in range(N_CORES)]
    for b in order:
        c = min(range(N_CORES), key=lambda k: loads[k])
        loads[c] += int(pads[b])
        core_batches[c].append(int(b))
    # phase 1: balance the PADDED loads (classic makespan objective)
    for _ in range(64):
        hi = max(range(N_CORES), key=lambda k: loads[k])
        best = None
        cur = (loads[hi], -min(loads))
        for lo in range(N_CORES):
            if lo == hi:
                continue
            for b in core_batches[hi]:
                nh, nl = loads[hi] - pads[b], loads[lo] + pads[b]
                cand = (max(nh, nl), -min(nh, nl))
                if cand < cur:
                    cur, best = cand, ("move", b, lo)
            for b in core_batches[hi]:
                for b2 in core_batches[lo]:
                    if pads[b] <= pads[b2]:
                        continue
                    nh = loads[hi] - pads[b] + pads[b2]
                    nl = loads[lo] + pads[b] - pads[b2]
                    cand = (max(nh, nl), -min(nh, nl))
                    if cand < cur:
                        cur, best = cand, ("swap", b, b2, lo)
        if best is None:
            break
        if best[0] == "move":
            _, b, lo = best
            core_batches[hi].remove(b)
            core_batches[lo].append(b)
        else:
            _, b, b2, lo = best
            core_batches[hi].remove(b)
            core_batches[lo].remove(b2)
            core_batches[hi].append(b2)
            core_batches[lo].append(b)
        loads = [int(sum(pads[b] for b in cb)) for cb in core_batches]
    # phase 2: refine on the EFFECTIVE loads (last batch unpadded)
    for _ in range(96):
        cur = score(core_batches)
        hi = max(range(N_CORES), key=lambda k: eff(core_batches[k]))
        best = None
        for lo in range(N_CORES):
            if lo == hi:
                continue
            for b in list(core_batches[hi]):
                nh = [x for x in core_batches[hi] if x != b]
                nl = core_batches[lo] + [b]
                cbs2 = list(core_batches)
                cbs2[hi], cbs2[lo] = nh, nl
                cand = score(cbs2)
                if cand < cur:
                    cur, best = cand, ("move", b, lo)
            for b in list(core_batches[hi]):
                for b2 in list(core_batches[lo]):
                    nh = [x for x in core_batches[hi] if x != b] + [b2]
                    nl = [x for x in core_batches[lo] if x != b2] + [b]
                    cbs2 = list(core_batches)
                    cbs2[hi], cbs2[lo] = nh, nl
                    cand = score(cbs2)
                    if cand < cur:
                        cur, best = cand, ("swap", b, b2, lo)
        if best is None:
            break
        if best[0] == "move":
            _, b, lo = best
            core_batches[hi].remove(b)
            core_batches[lo].append(b)
        else:
            _, b, b2, lo = best
            core_batches[hi].remove(b)
            core_batches[lo].remove(b2)
            core_batches[hi].append(b2)
            core_batches[lo].append(b)
        loads = [int(sum(pads[b] for b in cb)) for cb in core_batches]
    R = max(eff(cb) for cb in core_batches)
    # layout: per core, list of (batch, row_offset, valid_len, padded_len),
    # with the max-waste batch placed last so its padding falls off the end.
    layout = []
    for c in range(N_CORES):
        cb = sorted(core_batches[c], key=lambda b: int(waste[b]))
        cur = 0
        segs = []
        for b in cb:
            segs.append((b, cur, int(lens[b]), int(pads[b])))
            cur += int(pads[b])
        layout.append(segs)
    return R, layout


def _run(inputs, trace=False):
    if trace or os.environ.get("BASS_TRACE"):
        _ensure_trace_support()
    from concourse.bass_utils import run_bass_kernel_spmd

    hidden = np.ascontiguousarray(np.asarray(inputs["hidden"]), dtype=np.float32)
    enc = np.asarray(inputs["encoder_outputs"])
    W = np.ascontiguousarray(np.asarray(inputs["W"]), dtype=np.float32)
    bvec = np.ascontiguousarray(np.asarray(inputs["b"]), dtype=np.float32)
    v = np.ascontiguousarray(np.asarray(inputs["v"]), dtype=np.float32)
    src_len = np.asarray(inputs["src_len"])

    import ml_dtypes

    f8 = ml_dtypes.float8_e4m3

    # host-side: per-batch bias = hidden @ W[:D] + b   (0.4% of the FLOPs)
    bias_all = ((hidden @ W[:D]) + bvec[None, :]).astype(np.float32)  # [B, D]
    # W_e pre-scaled by 32 so its N(0, 1/32) entries use the e4m3 normal
    # range; the tanh activation applies 1/32 to the PSUM result.
    w_e8 = np.ascontiguousarray(W[D:] * np.float32(32.0)).astype(f8)  # [D, D]

    R, layout = _plan(src_len)
    n_sub = (R + SUB - 1) // SUB

    in_maps = []
    for c in range(N_CORES):
        et = np.zeros((D, R), dtype=np.float32)
        bt = np.zeros((D, n_sub), dtype=np.float32)
        for b, off, ln, pad in layout[c]:
            et[:, off : off + ln] = np.asarray(enc[b, :ln, :], dtype=np.float32).T
            s1 = min((off + pad + SUB - 1) // SUB, n_sub)
            bt[:, off // SUB : s1] = bias_all[b][:, None]
        in_maps.append(
            {
                "et": et.astype(f8),
                "wt": w_e8,
                "vt": v,
                "bt": bt,
                "ot": np.ones((128, 1), dtype=np.float32),
            }
        )

    if R not in _NC_CACHE:
        _NC_CACHE[R] = _build_bass(R)
    nc = _NC_CACHE[R]

    res = run_bass_kernel_spmd(nc, in_maps, list(range(N_CORES)), trace=trace)

    attn = np.zeros((B, 1, S), dtype=np.float32)
    for c in range(N_CORES):
        sc = res.results[c]["scores"][0]
        for b, off, ln, _pad in layout[c]:
            srow = sc[off : off + ln].astype(np.float32)
            m = srow.max()
            e = np.exp(srow - m, dtype=np.float32)
            attn[b, 0, :ln] = e / e.sum(dtype=np.float32)
    return attn, res


def kernel(**inputs):
    attn, _ = _run(inputs, trace=False)
    return attn



# revision 67
# speedup vs baseline: 1.0215x; 1.0071x over previous
"""Bahdanau attention kernel for Trainium2 (8 NeuronCores).

Reference computation (B=32, S=2048, D=1024):
    x      = concat([broadcast(hidden), encoder_outputs], -1)   # [B,S,2D]
    energy = tanh(x @ W + b)                                    # [B,S,D]
    scores = energy . v                                         # [B,S]
    attn   = softmax(mask(scores, src_len))                     # [B,1,S]

Key transformations:
  * x @ W = encoder_outputs @ W[D:] + (hidden @ W[:D]);  the hidden part is
    a tiny per-batch bias vector computed on the host and folded into the
    tanh's per-partition bias on the device.
  * rows with s >= src_len[b] are fully masked out of the softmax, so they
    are never computed: the host packs only the valid rows (padded to SUB
    per batch), load-balances batches across the 8 cores (LPT + swap local
    search -- every core executes the same SPMD program over R = max core
    load rows), and the device runs a dense kernel on the packed rows.
  * the big E @ W_e matmul runs in fp8 e4m3 with the DoubleRow perf mode
    (two K-planes per cycle -> 2x the f32r MAC rate).  W_e is pre-scaled
    by 32 on the host so its entries use the e4m3 normal range; the 1/32
    is folded into the tanh activation's input scale.  The quantization
    error lands at ~1.2e-2 on the final softmax (gate: 2e-2).
  * the device computes energy^T tiles [D_out=128, rows] in PSUM
    (W_e stationary, host-pre-transposed E^T streaming), applies
    tanh(+bias) on the scalar engine (SUB=256-wide instructions -- at 128
    the ACT engine's per-instruction overhead throttles the pipeline), and
    folds v in on the vector engine as two independent even/odd fused
    multiply-add chains (one serial chain paces the whole pipeline).  A
    deferred per-tile ones-matmul (emitted inside the NEXT tile's matmul
    stream so the in-order PE queue never stalls on it) reduces the two
    acc tiles to the row scores.  Masking + softmax run on the host
    (cheap, O(B*S)) because the packed segment boundaries differ per core.
  * startup: a chain of tiny matmuls on the runtime's own const APs plus a
    memset-fed dummy chain puts the PE to work ~0.5us in, which starts the
    HAM DMA un-throttle clock while the first W/E^T tiles stream in.
"""

import os
import sys

import numpy as np

for _p in ("/root/.axon_site/_ro/trn_rl_repo", "/opt/trn_rl_repo"):
    if os.path.isdir(_p) and _p not in sys.path:
        sys.path.append(_p)

B, S, D = 32, 2048, 1024
N_CORES = 8
# Per-batch row padding granularity == tanh bias subtile width.  256 wastes
# more rows than 128 (~5% vs ~2.5%) but halves the scalar engine's
# per-instruction overhead count: at 128 the ACT drain rate (~1032ns per
# 128x512 m-chunk) falls behind the PE fill rate (~862ns) and throttles the
# whole pipeline via PSUM-bank recycling.
SUB = 256
RB = 512  # main matmul row tile (PSUM free dim)
KO = D // 128  # K chunks (8)
MO = D // 128  # D_out chunks (8)

_NC_CACHE = {}


def _ensure_trace_support():
    """Make trace=True / BASS_TRACE=1 runs survive on images where
    ``antenv.axon_hooks`` is absent (the boot shim degrades silently but
    ``bass_utils`` imports it unconditionally) and where artifact uploads
    to remote storage are unavailable.  No-ops when everything exists."""
    import types

    try:
        import antenv

        try:
            import antenv.axon_hooks  # noqa: F401
        except ImportError:
            mod = types.ModuleType("antenv.axon_hooks")
            state = {"hook": None}
            mod.set_axon_ntff_profile_hook = lambda h: state.__setitem__("hook", h)
            mod.get_axon_ntff_profile_hook = lambda: state["hook"]
            sys.modules["antenv.axon_hooks"] = mod
            antenv.axon_hooks = mod
            try:
                from trn_agent_boot.trn_boot import _ntff_profile_via_ctypes

                so = "/opt/axon/libaxon_pjrt.so"
                if os.path.exists(so):
                    mod.set_axon_ntff_profile_hook(_ntff_profile_via_ctypes(so))
            except Exception:
                pass
    except Exception:
        pass
    try:
        import concourse.bass_utils as bu

        orig = bu.upload_artifacts
        if not getattr(orig, "_safe_wrapped", False):

            def _safe_upload(tmpdir, _orig=orig):
                try:
                    return _orig(tmpdir)
                except Exception:
                    return f"local://{tmpdir}"

            _safe_upload._safe_wrapped = True
            bu.upload_artifacts = _safe_upload
    except Exception:
        pass


def _row_tiles(R):
    """Row-tile sizes covering R rows: 512-tiles plus a 128/256/384 tail.

    Returns (sizes, row_offsets) in processing order."""
    assert R % 128 == 0
    sizes = [RB] * (R // RB)
    offs = [i * RB for i in range(len(sizes))]
    if R % RB:
        sizes.append(R % RB)
        offs.append((R // RB) * RB)
    return sizes, offs


def _build_bass(R):
    """Build the per-core SPMD program for R packed rows (R % 256 == 0)."""
    import concourse.bass as bass  # noqa: F401
    import concourse.tile as tile
    from concourse import bacc, mybir

    f32 = mybir.dt.float32
    f32r = mybir.dt.float32r
    f8 = mybir.dt.float8e4
    DR = mybir.MatmulPerfMode.DoubleRow
    n_sub = (R + SUB - 1) // SUB
    tiles, row_offs = _row_tiles(R)

    nc = bacc.Bacc()
    et_d = nc.dram_tensor("et", [D, R], f8, kind="ExternalInput")
    w_d = nc.dram_tensor("wt", [D, D], f8, kind="ExternalInput")
    v_d = nc.dram_tensor("vt", [D], f32, kind="ExternalInput")
    ones_d = nc.dram_tensor("ot", [128, 1], f32r, kind="ExternalInput")
    b_d = nc.dram_tensor("bt", [D, n_sub], f32, kind="ExternalInput")
    out_d = nc.dram_tensor("scores", [1, R], f32, kind="ExternalOutput")

    et_ap = et_d[:, :].rearrange("(ko p) r -> p ko r", p=128)
    w_ap = w_d[:, :].rearrange("(ko p) j -> p ko j", p=128)
    v_ap = v_d[:].rearrange("(mo p) -> p mo", p=128)
    b_ap = b_d[:, :].rearrange("(mo p) s -> p mo s", p=128)

    with tile.TileContext(nc) as tc:
        with (
            tc.tile_pool(name="singles", bufs=1) as singles,
            tc.tile_pool(name="warm", bufs=1) as warm,
            tc.tile_pool(name="et0", bufs=1) as et0_pool,
            tc.tile_pool(name="et", bufs=6) as et_pool,
            tc.tile_pool(name="tanh", bufs=8) as tanh_pool,
            tc.tile_pool(name="acc", bufs=3) as acc_pool,
            tc.tile_pool(name="sc", bufs=2) as sc_pool,
            tc.tile_pool(name="psum_e", bufs=7, space="PSUM") as psum_e,
            tc.tile_pool(name="psum_s", bufs=1, space="PSUM") as psum_s,
        ):
            # --- warmup: keep PE busy + load the ACT tanh table while the
            # first real DMAs are in flight (HAM un-throttles after ~3.4us
            # of PE activity; the ACT table load costs ~2.7us once).  The
            # dummy matmul chain ramps the PE p-state AND triggers the HAM
            # DMA un-throttle while the first W/E tiles stream in.
            # ultra-early PE activity: chain tiny matmuls on the const APs
            # that the runtime preamble loads anyway (~0.5us in), so the HAM
            # DMA un-throttle clock starts ~3us before the memset-fed warmup
            # below can.
            c1 = nc.const_aps.tensor(1.0, (128, 1), mybir.dt.float32)
            wps0 = psum_e.tile([128, RB], f32, tag="ps")
            for _ in range(40):
                nc.tensor.matmul(
                    wps0[0:1, 0:1], c1, c1, start=True, stop=True
                )
            wact = warm.tile([128, 2], f32)
            nc.vector.memset(wact[:], 0.0)
            nc.scalar.activation(
                out=wact[:, 1:2],
                in_=wact[:, 0:1],
                func=mybir.ActivationFunctionType.Tanh,
                bias=0.0,
                scale=1.0,
            )
            wdum = warm.tile([128, RB], f32)
            nc.vector.memset(wdum[:], 0.0)
            wps = psum_e.tile([128, RB], f32, tag="ps")
            for _ in range(5):
                nc.tensor.matmul(
                    wps[:, :],
                    wdum[:, 0:128].bitcast(f32r),
                    wdum[:, :].bitcast(f32r),
                    start=True,
                    stop=True,
                )

            # --- E^T row-block loads: two half-K DMAs per block so
            # dependencies unblock earlier.  Issued with a 2-block
            # prefetch depth; the first block is issued BEFORE the W
            # chunks so the pipeline can start as early as possible.

            et_tiles = {}

            def issue_et(rb):
                NT, r0 = tiles[rb], row_offs[rb]
                et_lo = et_pool.tile([128, KO // 2, RB], f8, tag="etl")
                et_hi = et_pool.tile([128, KO // 2, RB], f8, tag="eth")
                nc.sync.dma_start(
                    out=et_lo[:, :, :NT], in_=et_ap[:, : KO // 2, r0 : r0 + NT]
                )
                nc.sync.dma_start(
                    out=et_hi[:, :, :NT], in_=et_ap[:, KO // 2 :, r0 : r0 + NT]
                )
                et_tiles[rb] = (et_lo, et_hi)

            # --- rb 0 inputs: per-K-chunk E^T tiles interleaved with the W
            # chunk loads, so the first row block can compute k-progressively
            # while the 4MB of W is still arriving from HBM.
            NT0 = tiles[0]
            et0_p = []
            w_p = []
            for kp in range(KO // 2):
                t = et0_pool.tile([128, 2, RB], f8, tag=f"et0_{kp}")
                wk = singles.tile([128, 2, D], f8, tag=f"w{kp}")
                if kp == 0:
                    # first chunk split into singles: the very first matmul
                    # gates on less data
                    for kk in range(2):
                        nc.sync.dma_start(
                            out=t[:, kk, :NT0], in_=et_ap[:, kk, 0:NT0]
                        )
                        nc.sync.dma_start(
                            out=wk[:, kk, :], in_=w_ap[:, kk, :]
                        )
                else:
                    nc.sync.dma_start(
                        out=t[:, :, :NT0],
                        in_=et_ap[:, 2 * kp : 2 * kp + 2, 0:NT0],
                    )
                    nc.sync.dma_start(
                        out=wk[:], in_=w_ap[:, 2 * kp : 2 * kp + 2, :]
                    )
                et0_p.append(t)
                w_p.append(wk)
            v_sb = singles.tile([128, MO], f32)
            nc.sync.dma_start(out=v_sb[:], in_=v_ap)
            bias_sb = singles.tile([128, MO, n_sub], f32)
            nc.sync.dma_start(out=bias_sb[:], in_=b_ap)
            ones_sb = singles.tile([128, 1], f32r)
            nc.sync.dma_start(out=ones_sb[:], in_=ones_d[:, :])

            for _rb in (1, 2, 3, 4, 5):
                if len(tiles) > _rb:
                    issue_et(_rb)

            def act_and_vdot(ps, accs, m, NT, row0):
                # tanh on the scalar engine (per-SUB bias), then fold v in:
                #   acc += tanh(...) * v[m-chunk].
                # The serial acc chain paces the pipeline (~900ns/FMA), so it
                # is split into TWO independent even/odd chains (gpsimd cannot
                # run TensorScalarPtr on core v3, so both stay on DVE).  The PE sums the two acc tiles in the per-tile
                # ones-matmul epilogue.
                # f32r tanh output: the ACT engine writes bf16 ~60% slower,
                # and the ones-matmul needs a single-pass dtype anyway.
                th = tanh_pool.tile([128, RB], f32r)
                pos = 0
                while pos < NT:
                    w = min(SUB, NT - pos)
                    s_idx = (row0 + pos) // SUB
                    nc.scalar.activation(
                        out=th[:, pos : pos + w],
                        in_=ps[:, pos : pos + w],
                        func=mybir.ActivationFunctionType.Tanh,
                        bias=bias_sb[:, m, s_idx : s_idx + 1],
                        scale=1.0 / 32.0,
                    )
                    pos += w
                eng = nc.vector
                acc = accs[m % 2]
                if m < 2:
                    eng.tensor_scalar_mul(acc[:, :NT], th[:, :NT], v_sb[:, m : m + 1])
                else:
                    eng.scalar_tensor_tensor(
                        out=acc[:, :NT],
                        in0=th[:, :NT],
                        scalar=v_sb[:, m : m + 1],
                        in1=acc[:, :NT],
                        op0=mybir.AluOpType.mult,
                        op1=mybir.AluOpType.add,
                    )

            # Row-tile epilogues (ones-matmul -> copy -> DMA) are deferred by
            # one tile: the ones-matmul depends on the tile's full ACT+DVE
            # chain, which completes while the NEXT tile's E-matmuls run.
            # Emitting it inside the next tile's matmul stream keeps the
            # in-order PE queue from stalling on it.
            pending = []

            def flush_pending():
                while pending:
                    accs_p, NT_p, row0_p = pending.pop(0)
                    sc_ps = psum_s.tile([1, RB], f32)
                    for j in range(2):
                        nc.tensor.matmul(
                            sc_ps[:, :NT_p],
                            ones_sb[:, 0:1],
                            accs_p[j][:, :NT_p],
                            start=(j == 0),
                            stop=(j == 1),
                        )
                    sc_sb = sc_pool.tile([1, RB], f32)
                    nc.vector.tensor_copy(sc_sb[:, :NT_p], sc_ps[:, :NT_p])
                    nc.sync.dma_start(
                        out=out_d[0:1, row0_p : row0_p + NT_p],
                        in_=sc_sb[:, :NT_p],
                    )

            for rb, NT in enumerate(tiles):
                row0 = row_offs[rb]
                if rb + 6 < len(tiles):
                    issue_et(rb + 6)
                accs = (
                    acc_pool.tile([128, RB], f32r, name="acc_e", tag="acc_e"),
                    acc_pool.tile([128, RB], f32r, name="acc_o", tag="acc_o"),
                )
                if rb == 0:
                    # k-pair-outer halves: 4 open PSUM banks accumulate while
                    # the (w_kp, et0_kp) chunk pairs stream in.
                    for half in range(2):
                        ps_l = [
                            psum_e.tile(
                                [128, RB], f32, name=f"ps0_{half}_{mi}", tag="ps"
                            )
                            for mi in range(4)
                        ]
                        for kp in range(KO // 2):
                            for mi in range(4):
                                m = half * 4 + mi
                                nc.tensor.matmul(
                                    ps_l[mi][:, :NT],
                                    w_p[kp][:, :, m * 128 : (m + 1) * 128],
                                    et0_p[kp][:, :, :NT],
                                    start=(kp == 0),
                                    stop=(kp == KO // 2 - 1),
                                    perf_mode=DR,
                                )
                        for mi in range(4):
                            act_and_vdot(ps_l[mi], accs, half * 4 + mi, NT, row0)
                else:
                    et_half = et_tiles.pop(rb)
                    for m in range(MO):
                        ps = psum_e.tile([128, RB], f32, tag="ps")
                        for kp in range(KO // 2):
                            src = et_half[kp // 2]
                            ko2 = (kp % 2) * 2
                            nc.tensor.matmul(
                                ps[:, :NT],
                                w_p[kp][:, :, m * 128 : (m + 1) * 128],
                                src[:, ko2 : ko2 + 2, :NT],
                                start=(kp == 0),
                                stop=(kp == KO // 2 - 1),
                                perf_mode=DR,
                            )
                        if m == 2:
                            flush_pending()
                        act_and_vdot(ps, accs, m, NT, row0)
                pending.append((accs, NT, row0))
            flush_pending()
    nc.compile()
    return nc


def _plan(src_len):
    """Pack valid rows (padded to SUB per batch) and balance across cores.

    Every core executes the same SPMD program over R = max effective load,
    so shaving the max directly shaves kernel time.  The LAST batch on each
    core needs no SUB-rounding (nothing follows it), so a core's effective
    load is sum(pads of all but the max-waste batch) + true length of that
    batch, rounded to the 128-row tile granularity.  LPT greedy on padded
    sizes, then a move/swap local search on the effective objective."""
    lens = np.clip(np.asarray(src_len).astype(np.int64), 1, S)
    pads = ((lens + SUB - 1) // SUB) * SUB
    waste = pads - lens

    def eff(cb):
        if not cb:
            return 0
        load = int(sum(pads[b] for b in cb))
        w = max(int(waste[b]) for b in cb)
        return ((load - w + 127) // 128) * 128

    def score(cbs):
        effs = sorted((eff(cb) for cb in cbs), reverse=True)
        return tuple(effs)

    order = np.argsort(-pads, kind="stable")
    loads = [0] * N_CORES
    core_batches = [[] for _ in range(N_CORES)]
    for b in order:
        c = min(range(N_CORES), key=lambda k: loads[k])
        loads[c] += int(pads[b])
        core_batches[c].append(int(b))
    # phase 1: balance the PADDED loads (classic makespan objective)
    for _ in range(64):
        hi = max(range(N_CORES), key=lambda k: loads[k])
        best = None
        cur = (loads[hi], -min(loads))
        for lo in range(N_CORES):
            if lo == hi:
                continue
            for b in core_batches[hi]:
                nh, nl = loads[hi] - pads[b], loads[lo] + pads[b]
                cand = (max(nh, nl), -min(nh, nl))
                if cand < cur:
                    cur, best = cand, ("move", b, lo)
            for b in core_batches[hi]:
                for b2 in core_batches[lo]:
                    if pads[b] <= pads[b2]:
                        continue
                    nh = loads[hi] - pads[b] + pads[b2]
                    nl = loads[lo] + pads[b] - pads[b2]
                    cand = (max(nh, nl), -min(nh, nl))
                    if cand < cur:
                        cur, best = cand, ("swap", b, b2, lo)
        if best is None:
            break
        if best[0] == "move":
            _, b, lo = best
            core_batches[hi].remove(b)
            core_batches[lo].append(b)
        else:
            _, b, b2, lo = best
            core_batches[hi].remove(b)
            core_batches[lo].remove(b2)
            core_batches[hi].append(b2)
            core_batches[lo].append(b)
        loads = [int(sum(pads[b] for b in cb)) for cb in core_batches]
    # phase 2: refine on the EFFECTIVE loads (last batch unpadded)
    for _ in range(96):
        cur = score(core_batches)
        hi = max(range(N_CORES), key=lambda k: eff(core_batches[k]))
        best = None
        for lo in range(N_CORES):
            if lo == hi:
                continue
            for b in list(core_batches[hi]):
                nh = [x for x in core_batches[hi] if x != b]
                nl = core_batches[lo] + [b]
                cbs2 = list(core_batches)
                cbs2[hi], cbs2[lo] = nh, nl
                cand = score(cbs2)
                if cand < cur:
                    cur, best = cand, ("move", b, lo)
            for b in list(core_batches[hi]):
                for b2 in list(core_batches[lo]):
                    nh = [x for x in core_batches[hi] if x != b] + [b2]
                    nl = [x for x in core_batches[lo] if x != b2] + [b]
                    cbs2 = list(core_batches)
                    cbs2[hi], cbs2[lo] = nh, nl
                    cand = score(cbs2)
                    if cand < cur:
                        cur, best = cand, ("swap", b, b2, lo)
        if best is None:
            break
        if best[0] == "move":
            _, b, lo = best
            core_batches[hi].remove(b)
            core_batches[lo].append(b)
        else:
            _, b, b2, lo = best
            core_batches[hi].remove(b)
            core_batches[lo].remove(b2)
            core_batches[hi].append(b2)
            core_batches[lo].append(b)
        loads = [int(sum(pads[b] for b in cb)) for cb in core_batches]
    R = max(eff(cb) for cb in core_batches)
    # layout: per core, list of (batch, row_offset, valid_len, padded_len),
    # with the max-waste batch placed last so its padding falls off the end.
    layout = []
    for c in range(N_CORES):
        cb = sorted(core_batches[c], key=lambda b: int(waste[b]))
        cur = 0
        segs = []
        for b in cb:
            segs.append((b, cur, int(lens[b]), int(pads[b])))
            cur += int(pads[b])
        layout.append(segs)
    return R, layout


def _run(inputs, trace=False):
    if trace or os.environ.get("BASS_TRACE"):
        _ensure_trace_support()
    from concourse.bass_utils import run_bass_kernel_spmd

    hidden = np.ascontiguousarray(np.asarray(inputs["hidden"]), dtype=np.float32)
    enc = np.asarray(inputs["encoder_outputs"])
    W = np.ascontiguousarray(np.asarray(inputs["W"]), dtype=np.float32)
    bvec = np.ascontiguousarray(np.asarray(inputs["b"]), dtype=np.float32)
    v = np.ascontiguousarray(np.asarray(inputs["v"]), dtype=np.float32)
    src_len = np.asarray(inputs["src_len"])

    import ml_dtypes

    f8 = ml_dtypes.float8_e4m3

    # host-side: per-batch bias = hidden @ W[:D] + b   (0.4% of the FLOPs)
    bias_all = ((hidden @ W[:D]) + bvec[None, :]).astype(np.float32)  # [B, D]
    # W_e pre-scaled by 32 so its N(0, 1/32) entries use the e4m3 normal
    # range; the tanh activation applies 1/32 to the PSUM result.
    w_e8 = np.ascontiguousarray(W[D:] * np.float32(32.0)).astype(f8)  # [D, D]

    R, layout = _plan(src_len)
    n_sub = (R + SUB - 1) // SUB

    in_maps = []
    for c in range(N_CORES):
        et = np.zeros((D, R), dtype=np.float32)
        bt = np.zeros((D, n_sub), dtype=np.float32)
        for b, off, ln, pad in layout[c]:
            et[:, off : off + ln] = np.asarray(enc[b, :ln, :], dtype=np.float32).T
            s1 = min((off + pad + SUB - 1) // SUB, n_sub)
            bt[:, off // SUB : s1] = bias_all[b][:, None]
        in_maps.append(
            {
                "et": et.astype(f8),
                "wt": w_e8,
                "vt": v,
                "bt": bt,
                "ot": np.ones((128, 1), dtype=np.float32),
            }
        )

    if R not in _NC_CACHE:
        _NC_CACHE[R] = _build_bass(R)
    nc = _NC_CACHE[R]

    res = run_bass_kernel_spmd(nc, in_maps, list(range(N_CORES)), trace=trace)

    attn = np.zeros((B, 1, S), dtype=np.float32)
    for c in range(N_CORES):
        sc = res.results[c]["scores"][0]
        for b, off, ln, _pad in layout[c]:
            srow = sc[off : off + ln].astype(np.float32)
            m = srow.max()
            e = np.exp(srow - m, dtype=np.float32)
            attn[b, 0, :ln] = e / e.sum(dtype=np.float32)
    return attn, res


def kernel(**inputs):
    attn, _ = _run(inputs, trace=False)
    return attn



# revision 68
# speedup vs baseline: 1.0228x; 1.0013x over previous
"""Bahdanau attention kernel for Trainium2 (8 NeuronCores).

Reference computation (B=32, S=2048, D=1024):
    x      = concat([broadcast(hidden), encoder_outputs], -1)   # [B,S,2D]
    energy = tanh(x @ W + b)                                    # [B,S,D]
    scores = energy . v                                         # [B,S]
    attn   = softmax(mask(scores, src_len))                     # [B,1,S]

Key transformations:
  * x @ W = encoder_outputs @ W[D:] + (hidden @ W[:D]);  the hidden part is
    a tiny per-batch bias vector computed on the host and folded into the
    tanh's per-partition bias on the device.
  * rows with s >= src_len[b] are fully masked out of the softmax, so they
    are never computed: the host packs only the valid rows (padded to SUB
    per batch), load-balances batches across the 8 cores (LPT + swap local
    search -- every core executes the same SPMD program over R = max core
    load rows), and the device runs a dense kernel on the packed rows.
  * the big E @ W_e matmul runs in fp8 e4m3 with the DoubleRow perf mode
    (two K-planes per cycle -> 2x the f32r MAC rate).  W_e is pre-scaled
    by 32 on the host so its entries use the e4m3 normal range; the 1/32
    is folded into the tanh activation's input scale.  The quantization
    error lands at ~1.2e-2 on the final softmax (gate: 2e-2).
  * the device computes energy^T tiles [D_out=128, rows] in PSUM
    (W_e stationary, host-pre-transposed E^T streaming), applies
    tanh(+bias) on the scalar engine (SUB=256-wide instructions -- at 128
    the ACT engine's per-instruction overhead throttles the pipeline), and
    folds v in on the vector engine as two independent even/odd fused
    multiply-add chains (one serial chain paces the whole pipeline).  A
    deferred per-tile ones-matmul (emitted inside the NEXT tile's matmul
    stream so the in-order PE queue never stalls on it) reduces the two
    acc tiles to the row scores.  Masking + softmax run on the host
    (cheap, O(B*S)) because the packed segment boundaries differ per core.
  * startup: a chain of tiny matmuls on the runtime's own const APs plus a
    memset-fed dummy chain puts the PE to work ~0.5us in, which starts the
    HAM DMA un-throttle clock while the first W/E^T tiles stream in.
"""

import os
import sys

import numpy as np

for _p in ("/root/.axon_site/_ro/trn_rl_repo", "/opt/trn_rl_repo"):
    if os.path.isdir(_p) and _p not in sys.path:
        sys.path.append(_p)

B, S, D = 32, 2048, 1024
N_CORES = 8
# Per-batch row padding granularity == tanh bias subtile width.  256 wastes
# more rows than 128 (~5% vs ~2.5%) but halves the scalar engine's
# per-instruction overhead count: at 128 the ACT drain rate (~1032ns per
# 128x512 m-chunk) falls behind the PE fill rate (~862ns) and throttles the
# whole pipeline via PSUM-bank recycling.
SUB = 256
RB = 512  # main matmul row tile (PSUM free dim)
KO = D // 128  # K chunks (8)
MO = D // 128  # D_out chunks (8)

_NC_CACHE = {}


def _ensure_trace_support():
    """Make trace=True / BASS_TRACE=1 runs survive on images where
    ``antenv.axon_hooks`` is absent (the boot shim degrades silently but
    ``bass_utils`` imports it unconditionally) and where artifact uploads
    to remote storage are unavailable.  No-ops when everything exists."""
    import types

    try:
        import antenv

        try:
            import antenv.axon_hooks  # noqa: F401
        except ImportError:
            mod = types.ModuleType("antenv.axon_hooks")
            state = {"hook": None}
            mod.set_axon_ntff_profile_hook = lambda h: state.__setitem__("hook", h)
            mod.get_axon_ntff_profile_hook = lambda: state["hook"]
            sys.modules["antenv.axon_hooks"] = mod
            antenv.axon_hooks = mod
            try:
                from trn_agent_boot.trn_boot import _ntff_profile_via_ctypes

                so = "/opt/axon/libaxon_pjrt.so"
                if os.path.exists(so):
                    mod.set_axon_ntff_profile_hook(_ntff_profile_via_ctypes(so))
            except Exception:
                pass
    except Exception:
        pass
    try:
        import concourse.bass_utils as bu

        orig = bu.upload_artifacts
        if not getattr(orig, "_safe_wrapped", False):

            def _safe_upload(tmpdir, _orig=orig):
                try:
                    return _orig(tmpdir)
                except Exception:
                    return f"local://{tmpdir}"

            _safe_upload._safe_wrapped = True
            bu.upload_artifacts = _safe_upload
    except Exception:
        pass


def _row_tiles(R):
    """Row-tile sizes covering R rows: 512-tiles plus a 128/256/384 tail.

    Returns (sizes, row_offsets) in processing order."""
    assert R % 128 == 0
    sizes = [RB] * (R // RB)
    offs = [i * RB for i in range(len(sizes))]
    if R % RB:
        sizes.append(R % RB)
        offs.append((R // RB) * RB)
    return sizes, offs


def _build_bass(R):
    """Build the per-core SPMD program for R packed rows (R % 256 == 0)."""
    import concourse.bass as bass  # noqa: F401
    import concourse.tile as tile
    from concourse import bacc, mybir

    f32 = mybir.dt.float32
    f32r = mybir.dt.float32r
    f8 = mybir.dt.float8e4
    DR = mybir.MatmulPerfMode.DoubleRow
    n_sub = (R + SUB - 1) // SUB
    tiles, row_offs = _row_tiles(R)

    nc = bacc.Bacc()
    et_d = nc.dram_tensor("et", [D, R], f8, kind="ExternalInput")
    w_d = nc.dram_tensor("wt", [D, D], f8, kind="ExternalInput")
    v_d = nc.dram_tensor("vt", [D], f32, kind="ExternalInput")
    ones_d = nc.dram_tensor("ot", [128, 1], f32r, kind="ExternalInput")
    b_d = nc.dram_tensor("bt", [D, n_sub], f32, kind="ExternalInput")
    out_d = nc.dram_tensor("scores", [1, R], f32, kind="ExternalOutput")

    et_ap = et_d[:, :].rearrange("(ko p) r -> p ko r", p=128)
    w_ap = w_d[:, :].rearrange("(ko p) j -> p ko j", p=128)
    v_ap = v_d[:].rearrange("(mo p) -> p mo", p=128)
    b_ap = b_d[:, :].rearrange("(mo p) s -> p mo s", p=128)

    with tile.TileContext(nc) as tc:
        with (
            tc.tile_pool(name="singles", bufs=1) as singles,
            tc.tile_pool(name="warm", bufs=1) as warm,
            tc.tile_pool(name="et0", bufs=1) as et0_pool,
            tc.tile_pool(name="et", bufs=6) as et_pool,
            tc.tile_pool(name="tanh", bufs=8) as tanh_pool,
            tc.tile_pool(name="acc", bufs=3) as acc_pool,
            tc.tile_pool(name="sc", bufs=2) as sc_pool,
            tc.tile_pool(name="psum_e", bufs=7, space="PSUM") as psum_e,
            tc.tile_pool(name="psum_s", bufs=1, space="PSUM") as psum_s,
        ):
            # --- warmup: keep PE busy + load the ACT tanh table while the
            # first real DMAs are in flight (HAM un-throttles after ~3.4us
            # of PE activity; the ACT table load costs ~2.7us once).  The
            # dummy matmul chain ramps the PE p-state AND triggers the HAM
            # DMA un-throttle while the first W/E tiles stream in.
            # ultra-early PE activity: chain tiny matmuls on the const APs
            # that the runtime preamble loads anyway (~0.5us in), so the HAM
            # DMA un-throttle clock starts ~3us before the memset-fed warmup
            # below can.
            c1 = nc.const_aps.tensor(1.0, (128, 1), mybir.dt.float32)
            wps0 = psum_e.tile([128, RB], f32, tag="ps")
            for _ in range(40):
                nc.tensor.matmul(
                    wps0[0:1, 0:1], c1, c1, start=True, stop=True
                )
            wact = warm.tile([128, 2], f32)
            nc.vector.memset(wact[:], 0.0)
            nc.scalar.activation(
                out=wact[:, 1:2],
                in_=wact[:, 0:1],
                func=mybir.ActivationFunctionType.Tanh,
                bias=0.0,
                scale=1.0,
            )

            # --- E^T row-block loads: two half-K DMAs per block so
            # dependencies unblock earlier.  Issued with a 2-block
            # prefetch depth; the first block is issued BEFORE the W
            # chunks so the pipeline can start as early as possible.

            et_tiles = {}

            def issue_et(rb):
                NT, r0 = tiles[rb], row_offs[rb]
                et_lo = et_pool.tile([128, KO // 2, RB], f8, tag="etl")
                et_hi = et_pool.tile([128, KO // 2, RB], f8, tag="eth")
                nc.sync.dma_start(
                    out=et_lo[:, :, :NT], in_=et_ap[:, : KO // 2, r0 : r0 + NT]
                )
                nc.sync.dma_start(
                    out=et_hi[:, :, :NT], in_=et_ap[:, KO // 2 :, r0 : r0 + NT]
                )
                et_tiles[rb] = (et_lo, et_hi)

            # --- rb 0 inputs: per-K-chunk E^T tiles interleaved with the W
            # chunk loads, so the first row block can compute k-progressively
            # while the 4MB of W is still arriving from HBM.
            NT0 = tiles[0]
            et0_p = []
            w_p = []
            for kp in range(KO // 2):
                t = et0_pool.tile([128, 2, RB], f8, tag=f"et0_{kp}")
                wk = singles.tile([128, 2, D], f8, tag=f"w{kp}")
                if kp == 0:
                    # first chunk split into singles: the very first matmul
                    # gates on less data
                    for kk in range(2):
                        nc.sync.dma_start(
                            out=t[:, kk, :NT0], in_=et_ap[:, kk, 0:NT0]
                        )
                        nc.sync.dma_start(
                            out=wk[:, kk, :], in_=w_ap[:, kk, :]
                        )
                else:
                    nc.sync.dma_start(
                        out=t[:, :, :NT0],
                        in_=et_ap[:, 2 * kp : 2 * kp + 2, 0:NT0],
                    )
                    nc.sync.dma_start(
                        out=wk[:], in_=w_ap[:, 2 * kp : 2 * kp + 2, :]
                    )
                et0_p.append(t)
                w_p.append(wk)
            v_sb = singles.tile([128, MO], f32)
            nc.sync.dma_start(out=v_sb[:], in_=v_ap)
            bias_sb = singles.tile([128, MO, n_sub], f32)
            nc.sync.dma_start(out=bias_sb[:], in_=b_ap)
            ones_sb = singles.tile([128, 1], f32r)
            nc.sync.dma_start(out=ones_sb[:], in_=ones_d[:, :])

            for _rb in (1, 2, 3, 4, 5):
                if len(tiles) > _rb:
                    issue_et(_rb)

            def act_and_vdot(ps, accs, m, NT, row0):
                # tanh on the scalar engine (per-SUB bias), then fold v in:
                #   acc += tanh(...) * v[m-chunk].
                # The serial acc chain paces the pipeline (~900ns/FMA), so it
                # is split into TWO independent even/odd chains (gpsimd cannot
                # run TensorScalarPtr on core v3, so both stay on DVE).  The PE sums the two acc tiles in the per-tile
                # ones-matmul epilogue.
                # f32r tanh output: the ACT engine writes bf16 ~60% slower,
                # and the ones-matmul needs a single-pass dtype anyway.
                th = tanh_pool.tile([128, RB], f32r)
                pos = 0
                while pos < NT:
                    w = min(SUB, NT - pos)
                    s_idx = (row0 + pos) // SUB
                    nc.scalar.activation(
                        out=th[:, pos : pos + w],
                        in_=ps[:, pos : pos + w],
                        func=mybir.ActivationFunctionType.Tanh,
                        bias=bias_sb[:, m, s_idx : s_idx + 1],
                        scale=1.0 / 32.0,
                    )
                    pos += w
                eng = nc.vector
                acc = accs[m % 2]
                if m < 2:
                    eng.tensor_scalar_mul(acc[:, :NT], th[:, :NT], v_sb[:, m : m + 1])
                else:
                    eng.scalar_tensor_tensor(
                        out=acc[:, :NT],
                        in0=th[:, :NT],
                        scalar=v_sb[:, m : m + 1],
                        in1=acc[:, :NT],
                        op0=mybir.AluOpType.mult,
                        op1=mybir.AluOpType.add,
                    )

            # Row-tile epilogues (ones-matmul -> copy -> DMA) are deferred by
            # one tile: the ones-matmul depends on the tile's full ACT+DVE
            # chain, which completes while the NEXT tile's E-matmuls run.
            # Emitting it inside the next tile's matmul stream keeps the
            # in-order PE queue from stalling on it.
            pending = []

            def flush_pending():
                while pending:
                    accs_p, NT_p, row0_p = pending.pop(0)
                    sc_ps = psum_s.tile([1, RB], f32)
                    for j in range(2):
                        nc.tensor.matmul(
                            sc_ps[:, :NT_p],
                            ones_sb[:, 0:1],
                            accs_p[j][:, :NT_p],
                            start=(j == 0),
                            stop=(j == 1),
                        )
                    sc_sb = sc_pool.tile([1, RB], f32)
                    nc.vector.tensor_copy(sc_sb[:, :NT_p], sc_ps[:, :NT_p])
                    nc.sync.dma_start(
                        out=out_d[0:1, row0_p : row0_p + NT_p],
                        in_=sc_sb[:, :NT_p],
                    )

            for rb, NT in enumerate(tiles):
                row0 = row_offs[rb]
                if rb + 6 < len(tiles):
                    issue_et(rb + 6)
                accs = (
                    acc_pool.tile([128, RB], f32r, name="acc_e", tag="acc_e"),
                    acc_pool.tile([128, RB], f32r, name="acc_o", tag="acc_o"),
                )
                if rb == 0:
                    # k-pair-outer halves: 4 open PSUM banks accumulate while
                    # the (w_kp, et0_kp) chunk pairs stream in.
                    for half in range(2):
                        ps_l = [
                            psum_e.tile(
                                [128, RB], f32, name=f"ps0_{half}_{mi}", tag="ps"
                            )
                            for mi in range(4)
                        ]
                        for kp in range(KO // 2):
                            for mi in range(4):
                                m = half * 4 + mi
                                nc.tensor.matmul(
                                    ps_l[mi][:, :NT],
                                    w_p[kp][:, :, m * 128 : (m + 1) * 128],
                                    et0_p[kp][:, :, :NT],
                                    start=(kp == 0),
                                    stop=(kp == KO // 2 - 1),
                                    perf_mode=DR,
                                )
                        for mi in range(4):
                            act_and_vdot(ps_l[mi], accs, half * 4 + mi, NT, row0)
                else:
                    et_half = et_tiles.pop(rb)
                    for m in range(MO):
                        ps = psum_e.tile([128, RB], f32, tag="ps")
                        for kp in range(KO // 2):
                            src = et_half[kp // 2]
                            ko2 = (kp % 2) * 2
                            nc.tensor.matmul(
                                ps[:, :NT],
                                w_p[kp][:, :, m * 128 : (m + 1) * 128],
                                src[:, ko2 : ko2 + 2, :NT],
                                start=(kp == 0),
                                stop=(kp == KO // 2 - 1),
                                perf_mode=DR,
                            )
                        if m == 2:
                            flush_pending()
                        act_and_vdot(ps, accs, m, NT, row0)
                pending.append((accs, NT, row0))
            flush_pending()
    nc.compile()
    return nc


def _plan(src_len):
    """Pack valid rows (padded to SUB per batch) and balance across cores.

    Every core executes the same SPMD program over R = max effective load,
    so shaving the max directly shaves kernel time.  The LAST batch on each
    core needs no SUB-rounding (nothing follows it), so a core's effective
    load is sum(pads of all but the max-waste batch) + true length of that
    batch, rounded to the 128-row tile granularity.  LPT greedy on padded
    sizes, then a move/swap local search on the effective objective."""
    lens = np.clip(np.asarray(src_len).astype(np.int64), 1, S)
    pads = ((lens + SUB - 1) // SUB) * SUB
    waste = pads - lens

    def eff(cb):
        if not cb:
            return 0
        load = int(sum(pads[b] for b in cb))
        w = max(int(waste[b]) for b in cb)
        return ((load - w + 127) // 128) * 128

    def score(cbs):
        effs = sorted((eff(cb) for cb in cbs), reverse=True)
        return tuple(effs)

    order = np.argsort(-pads, kind="stable")
    loads = [0] * N_CORES
    core_batches = [[] for _ in range(N_CORES)]
    for b in order:
        c = min(range(N_CORES), key=lambda k: loads[k])
        loads[c] += int(pads[b])
        core_batches[c].append(int(b))
    # phase 1: balance the PADDED loads (classic makespan objective)
    for _ in range(64):
        hi = max(range(N_CORES), key=lambda k: loads[k])
        best = None
        cur = (loads[hi], -min(loads))
        for lo in range(N_CORES):
            if lo == hi:
                continue
            for b in core_batches[hi]:
                nh, nl = loads[hi] - pads[b], loads[lo] + pads[b]
                cand = (max(nh, nl), -min(nh, nl))
                if cand < cur:
                    cur, best = cand, ("move", b, lo)
            for b in core_batches[hi]:
                for b2 in core_batches[lo]:
                    if pads[b] <= pads[b2]:
                        continue
                    nh = loads[hi] - pads[b] + pads[b2]
                    nl = loads[lo] + pads[b] - pads[b2]
                    cand = (max(nh, nl), -min(nh, nl))
                    if cand < cur:
                        cur, best = cand, ("swap", b, b2, lo)
        if best is None:
            break
        if best[0] == "move":
            _, b, lo = best
            core_batches[hi].remove(b)
            core_batches[lo].append(b)
        else:
            _, b, b2, lo = best
            core_batches[hi].remove(b)
            core_batches[lo].remove(b2)
            core_batches[hi].append(b2)
            core_batches[lo].append(b)
        loads = [int(sum(pads[b] for b in cb)) for cb in core_batches]
    # phase 2: refine on the EFFECTIVE loads (last batch unpadded)
    for _ in range(96):
        cur = score(core_batches)
        hi = max(range(N_CORES), key=lambda k: eff(core_batches[k]))
        best = None
        for lo in range(N_CORES):
            if lo == hi:
                continue
            for b in list(core_batches[hi]):
                nh = [x for x in core_batches[hi] if x != b]
                nl = core_batches[lo] + [b]
                cbs2 = list(core_batches)
                cbs2[hi], cbs2[lo] = nh, nl
                cand = score(cbs2)
                if cand < cur:
                    cur, best = cand, ("move", b, lo)
            for b in list(core_batches[hi]):
                for b2 in list(core_batches[lo]):
                    nh = [x for x in core_batches[hi] if x != b] + [b2]
                    nl = [x for x in core_batches[lo] if x != b2] + [b]
                    cbs2 = list(core_batches)
                    cbs2[hi], cbs2[lo] = nh, nl
                    cand = score(cbs2)
                    if cand < cur:
                        cur, best = cand, ("swap", b, b2, lo)
        if best is None:
            break
        if best[0] == "move":
            _, b, lo = best
            core_batches[hi].remove(b)
            core_batches[lo].append(b)
        else:
            _, b, b2, lo = best
            core_batches[hi].remove(b)
            core_batches[lo].remove(b2)
            core_batches[hi].append(b2)
            core_batches[lo].append(b)
        loads = [int(sum(pads[b] for b in cb)) for cb in core_batches]
    R = max(eff(cb) for cb in core_batches)
    # layout: per core, list of (batch, row_offset, valid_len, padded_len),
    # with the max-waste batch placed last so its padding falls off the end.
    layout = []
    for c in range(N_CORES):
        cb = sorted(core_batches[c], key=lambda b: int(waste[b]))
        cur = 0
        segs = []
        for b in cb:
            segs.append((b, cur, int(lens[b]), int(pads[b])))
            cur += int(pads[b])
        layout.append(segs)
    return R, layout


def _run(inputs, trace=False):
    if trace or os.environ.get("BASS_TRACE"):
        _ensure_trace_support()
    from concourse.bass_utils import run_bass_kernel_spmd

    hidden = np.ascontiguousarray(np.asarray(inputs["hidden"]), dtype=np.float32)
    enc = np.asarray(inputs["encoder_outputs"])
    W = np.ascontiguousarray(np.asarray(inputs["W"]), dtype=np.float32)
    bvec = np.ascontiguousarray(np.asarray(inputs["b"]), dtype=np.float32)
    v = np.ascontiguousarray(np.asarray(inputs["v"]), dtype=np.float32)
    src_len = np.asarray(inputs["src_len"])

    import ml_dtypes

    f8 = ml_dtypes.float8_e4m3

    # host-side: per-batch bias = hidden @ W[:D] + b   (0.4% of the FLOPs)
    bias_all = ((hidden @ W[:D]) + bvec[None, :]).astype(np.float32)  # [B, D]
    # W_e pre-scaled by 32 so its N(0, 1/32) entries use the e4m3 normal
    # range; the tanh activation applies 1/32 to the PSUM result.
    w_e8 = np.ascontiguousarray(W[D:] * np.float32(32.0)).astype(f8)  # [D, D]

    R, layout = _plan(src_len)
    n_sub = (R + SUB - 1) // SUB

    in_maps = []
    for c in range(N_CORES):
        et = np.zeros((D, R), dtype=np.float32)
        bt = np.zeros((D, n_sub), dtype=np.float32)
        for b, off, ln, pad in layout[c]:
            et[:, off : off + ln] = np.asarray(enc[b, :ln, :], dtype=np.float32).T
            s1 = min((off + pad + SUB - 1) // SUB, n_sub)
            bt[:, off // SUB : s1] = bias_all[b][:, None]
        in_maps.append(
            {
                "et": et.astype(f8),
                "wt": w_e8,
                "vt": v,
                "bt": bt,
                "ot": np.ones((128, 1), dtype=np.float32),
            }
        )

    if R not in _NC_CACHE:
        _NC_CACHE[R] = _build_bass(R)
    nc = _NC_CACHE[R]

    res = run_bass_kernel_spmd(nc, in_maps, list(range(N_CORES)), trace=trace)

    attn = np.zeros((B, 1, S), dtype=np.float32)
    for c in range(N_CORES):
        sc = res.results[c]["scores"][0]
        for b, off, ln, _pad in layout[c]:
            srow = sc[off : off + ln].astype(np.float32)
            m = srow.max()
            e = np.exp(srow - m, dtype=np.float32)
            attn[b, 0, :ln] = e / e.sum(dtype=np.float32)
    return attn, res


def kernel(**inputs):
    attn, _ = _run(inputs, trace=False)
    return attn

